# revision 1
# baseline (speedup 1.0000x reference)
"""ACM-GCN (2-layer) distributed Bass kernel for 8 TRN2 NeuronCores.

Strategy:
  - Shard nodes (rows of x / segment ids) across 8 cores: core k owns global
    rows [k*6250, (k+1)*6250), padded to 6272 = 49*128 per core.
  - Layer matmuls (x @ W_*) are local per core (lhsT = pre-transposed x tiles
    shipped from host in bf16).
  - SpMM (adj_low @ xl etc): each core owns the edges whose DESTINATION row
    lives on it. The [xl|xh] activations are AllGathered into a per-core HBM
    table; per 128-row destination window the kernel dma_gathers the source
    rows (bf16, 512B rows) and segment-sums them with one-hot val matmuls
    accumulated in PSUM (lhsT = one-hot(val) built on DVE with a fused
    tensor_scalar is_equal*mult).
  - Attention + combine + log_softmax are node-parallel (local).

All metadata (edge->window assignment, gather indices, one-hot row ids and
vals) is packed host-side in numpy; the Bass graph is static with
data-dependent chunk capacities shared across all 8 cores (max over cores).
"""

import math

import numpy as np
import ml_dtypes

import concourse.bass as bass
import concourse.mybir as mybir
import concourse.tile as tile
from concourse import bacc
from concourse.masks import make_identity

F32 = mybir.dt.float32
BF16 = mybir.dt.bfloat16
I16 = mybir.dt.int16
BF = ml_dtypes.bfloat16
AF = mybir.ActivationFunctionType
ALU = mybir.AluOpType
AX = mybir.AxisListType

DEFAULT_CFG = dict(N=50000, F=512, H=128, C=64, NC=8, GROUP=4, SPLIT=2, NQ=4)


# --------------------------------------------------------------------------
# Host-side planning / packing
# --------------------------------------------------------------------------

def derive(cfg):
    N, F, H, C, NC = cfg["N"], cfg["F"], cfg["H"], cfg["C"], cfg["NC"]
    assert N % NC == 0
    NSH = N // NC
    PW = (NSH + 127) // 128
    NPAD = PW * 128
    TBL = NC * NPAD
    HALF = TBL // 2
    assert HALF <= 32768, "int16 gather index limit"
    assert F % 128 == 0
    FK = F // 128
    return dict(NSH=NSH, PW=PW, NPAD=NPAD, TBL=TBL, HALF=HALF, FK=FK)


def make_plan(edge_row, edge_col, cfg):
    """Static shape plan shared by all cores: chunk counts per (window, half),
    grouping, chunk bases."""
    d = derive(cfg)
    NC, GROUP = cfg["NC"], cfg["GROUP"]
    NSH, PW, NPAD, HALF = d["NSH"], d["PW"], d["NPAD"], d["HALF"]

    core = edge_row // NSH
    dr = edge_row - core * NSH
    w = dr // 128
    tcol = (edge_col // NSH) * NPAD + (edge_col % NSH)
    half = (tcol >= HALF).astype(np.int64)
    key = (core.astype(np.int64) * PW + w) * 2 + half
    counts = np.bincount(key, minlength=NC * PW * 2).reshape(NC, PW, 2)
    gchunks = (counts.max(axis=0) + 127) // 128  # [PW, 2]

    groups = [list(range(g, min(g + GROUP, PW))) for g in range(0, PW, GROUP)]
    chunk_base = {}
    gc = 0
    calls = []  # per group: [(half, total_chunks, base_chunk)]
    for ws in groups:
        gcalls = []
        for hf in (0, 1):
            base = gc
            for wi in ws:
                chunk_base[(wi, hf)] = gc
                gc += int(gchunks[wi, hf])
            gcalls.append((hf, gc - base, base))
        calls.append(gcalls)
    return dict(d=d, gchunks=gchunks, groups=groups, chunk_base=chunk_base,
                calls=calls, GC=gc)


def plan_key(plan, cfg):
    return (tuple(sorted(cfg.items())),
            tuple(map(int, plan["gchunks"].flatten())))


def pack_inputs(inputs, plan, cfg):
    """Build per-core in_maps (numpy) for the bass program."""
    d = plan["d"]
    N, F, H, C, NC = cfg["N"], cfg["F"], cfg["H"], cfg["C"], cfg["NC"]
    NSH, PW, NPAD, HALF, FK = d["NSH"], d["PW"], d["NPAD"], d["HALF"], d["FK"]
    GC = plan["GC"]
    chunk_base = plan["chunk_base"]

    x = np.asarray(inputs["x"], np.float32)
    er = np.asarray(inputs["edge_row"]).astype(np.int64)
    ec = np.asarray(inputs["edge_col"]).astype(np.int64)
    ev = np.asarray(inputs["edge_val"], np.float32)

    w1 = np.concatenate([np.asarray(inputs["weight_low"], np.float32),
                         np.asarray(inputs["weight_high"], np.float32),
                         np.asarray(inputs["weight_mlp"], np.float32)], axis=1)  # [F, 3H]
    w2 = np.concatenate([np.asarray(inputs["weight_low2"], np.float32),
                         np.asarray(inputs["weight_high2"], np.float32),
                         np.asarray(inputs["weight_mlp2"], np.float32)], axis=1)  # [H, 3C]
    # w1 sbuf layout [128, FK, 3H]: [p, kk, n] = w1[kk*128+p, n]
    w1_sb = np.ascontiguousarray(
        w1.reshape(FK, 128, 3 * H).transpose(1, 0, 2)).astype(BF)
    w2_sb = w2.astype(BF)  # [H=128, 3C]

    attl1 = np.stack([np.asarray(inputs["att_vec_low"], np.float32)[:, 0],
                      np.asarray(inputs["att_vec_high"], np.float32)[:, 0],
                      np.asarray(inputs["att_vec_mlp"], np.float32)[:, 0]])  # [3, H]
    attl2 = np.stack([np.asarray(inputs["att_vec_low2"], np.float32)[:, 0],
                      np.asarray(inputs["att_vec_high2"], np.float32)[:, 0],
                      np.asarray(inputs["att_vec_mlp2"], np.float32)[:, 0]])  # [3, C]
    attl1_sb = np.broadcast_to(attl1[None], (128, 3, H)).astype(BF).copy()
    attl2_sb = np.broadcast_to(attl2[None], (128, 3, C)).astype(BF).copy()
    avec1 = (np.asarray(inputs["att_vec"], np.float32) / 3.0).reshape(9)
    avec2 = (np.asarray(inputs["att_vec2"], np.float32) / 3.0).reshape(9)
    avec1_sb = np.broadcast_to(avec1[None], (128, 9)).astype(np.float32).copy()
    avec2_sb = np.broadcast_to(avec2[None], (128, 9)).astype(np.float32).copy()
    colidx = np.broadcast_to(np.arange(128, dtype=np.float32)[None], (128, 128)
                             ).astype(BF).copy()

    # edge metadata
    core = er // NSH
    dr = er - core * NSH
    w = dr // 128
    rl = (dr % 128).astype(np.float32)
    tcol = (ec // NSH) * NPAD + (ec % NSH)
    half = (tcol >= HALF).astype(np.int64)
    idxv = (tcol - half * HALF).astype(np.int64)

    in_maps = []
    for k in range(NC):
        sel = core == k
        kw, khalf, kidx, krl, kval = w[sel], half[sel], idxv[sel], rl[sel], ev[sel]
        # order edges by (w, half)
        order = np.lexsort((khalf, kw))
        kw, khalf, kidx, krl, kval = (a[order] for a in (kw, khalf, kidx, krl, kval))
        seg_key = kw * 2 + khalf
        seg_counts = np.bincount(seg_key, minlength=PW * 2).reshape(PW, 2)
        # global slot for each edge: chunk_base[(w,half)]*128 + position-in-segment
        # position-in-segment:
        starts = np.zeros(PW * 2, np.int64)
        np.cumsum(seg_counts.flatten()[:-1], out=starts[1:])
        pos_in_seg = np.arange(len(kw)) - starts[seg_key]
        cb = np.array([[chunk_base[(wi, hf)] for hf in (0, 1)] for wi in range(PW)],
                      np.int64)
        slot = cb[kw, khalf] * 128 + (pos_in_seg // 128) * 128 + pos_in_seg % 128
        # (pos_in_seg fits within gchunks[w,half]*128 by construction)

        idx_flat = np.zeros(GC * 128, np.int16)
        rl_arr = np.zeros((128, GC), np.float32)
        val_arr = np.zeros((128, GC), np.float32)
        idx_flat[slot] = kidx.astype(np.int16)
        p = slot % 128
        c = slot // 128
        rl_arr[p, c] = krl
        val_arr[p, c] = kval
        # idxs layout for dma_gather: [j%16, j//16] replicated over 8 groups of
        # 16 partitions
        idx16 = np.tile(idx_flat.reshape(-1, 16).T, (8, 1))  # [128, GC*8]

        # pre-transposed x tiles: [PW, 128, FK, 128]:
        # xt[m, p, kk, j] = x[k*NSH + m*128 + j, kk*128 + p]
        xk = np.zeros((NPAD, F), np.float32)
        xk[:NSH] = x[k * NSH:(k + 1) * NSH]
        xt = np.ascontiguousarray(
            xk.reshape(PW, 128, FK, 128).transpose(0, 3, 2, 1)).astype(BF)
        # check: xt[m, pfeat, kk, jnode] = xk[m*128 + jnode, kk*128 + pfeat]

        in_maps.append({
            "xt": xt,
            "w1s": w1_sb, "w2s": w2_sb,
            "attl1": attl1_sb, "attl2": attl2_sb,
            "avec1": avec1_sb, "avec2": avec2_sb,
            "colidx": colidx,
            "idx16": idx16,
            "rloc": rl_arr,
            "vals": val_arr,
        })
    return in_maps


# --------------------------------------------------------------------------
# Bass program
# --------------------------------------------------------------------------

def build_program(plan, cfg, repeat=1):
    d = plan["d"]
    N, F, H, C, NC = cfg["N"], cfg["F"], cfg["H"], cfg["C"], cfg["NC"]
    NSH, PW, NPAD, TBL, HALF, FK = (d["NSH"], d["PW"], d["NPAD"], d["TBL"],
                                    d["HALF"], d["FK"])
    GC = plan["GC"]
    gchunks = plan["gchunks"]
    groups = plan["groups"]
    chunk_base = plan["chunk_base"]
    calls = plan["calls"]
    H2, C2, C3 = 2 * H, 2 * C, 3 * C
    H3 = 3 * H

    nc = bacc.Bacc(None, target_bir_lowering=False, num_devices=NC,
                   num_swdge_queues=int(cfg.get("NQ", 1)))

    xt_d = nc.declare_dram_parameter("xt", [PW, 128, FK, 128], BF16, isOutput=False)
    w1_d = nc.declare_dram_parameter("w1s", [128, FK, H3], BF16, isOutput=False)
    w2_d = nc.declare_dram_parameter("w2s", [H, C3], BF16, isOutput=False)
    attl1_d = nc.declare_dram_parameter("attl1", [128, 3, H], BF16, isOutput=False)
    attl2_d = nc.declare_dram_parameter("attl2", [128, 3, C], BF16, isOutput=False)
    avec1_d = nc.declare_dram_parameter("avec1", [128, 9], F32, isOutput=False)
    avec2_d = nc.declare_dram_parameter("avec2", [128, 9], F32, isOutput=False)
    colidx_d = nc.declare_dram_parameter("colidx", [128, 128], BF16, isOutput=False)
    idx16_d = nc.declare_dram_parameter("idx16", [128, GC * 8], I16, isOutput=False)
    rloc_d = nc.declare_dram_parameter("rloc", [128, GC], F32, isOutput=False)
    vals_d = nc.declare_dram_parameter("vals", [128, GC], F32, isOutput=False)
    out_d = nc.declare_dram_parameter("out", [NPAD, C], F32, isOutput=True)

    with tile.TileContext(nc) as tc:
        # ---- DRAM internals
        from contextlib import ExitStack
        es = ExitStack()
        dram_pool = es.enter_context(
            tc.tile_pool(name="dram_pool", bufs=1, space="DRAM"))
        t1_local = dram_pool.tile([NPAD, H2], BF16, name="t1_local", tag="t1l")
        t1_full = dram_pool.tile([TBL, H2], BF16, name="t1_full", tag="t1f")
        t2_local = dram_pool.tile([NPAD, C2], BF16, name="t2_local", tag="t2l")
        t2_full = dram_pool.tile([TBL, C2], BF16, name="t2_full", tag="t2f")

        # ---- resident constants / metadata
        consts = es.enter_context(tc.tile_pool(name="consts", bufs=1))
        w1_sb = consts.tile([128, FK, H3], BF16, name="w1_sb")
        nc.sync.dma_start(w1_sb[:], w1_d[:])
        w2_sb = consts.tile([H, C3], BF16, name="w2_sb")
        nc.sync.dma_start(w2_sb[:], w2_d[:])
        attl1_sb = consts.tile([128, 3, H], BF16, name="attl1_sb")
        nc.sync.dma_start(attl1_sb[:], attl1_d[:])
        attl2_sb = consts.tile([128, 3, C], BF16, name="attl2_sb")
        nc.sync.dma_start(attl2_sb[:], attl2_d[:])
        avec1_sb = consts.tile([128, 9], F32, name="avec1_sb")
        nc.sync.dma_start(avec1_sb[:], avec1_d[:])
        avec2_sb = consts.tile([128, 9], F32, name="avec2_sb")
        nc.sync.dma_start(avec2_sb[:], avec2_d[:])
        colidx_sb = consts.tile([128, 128], BF16, name="colidx_sb")
        nc.sync.dma_start(colidx_sb[:], colidx_d[:])
        idx_sb = consts.tile([128, GC * 8], I16, name="idx_sb")
        nc.sync.dma_start(idx_sb[:], idx16_d[:])
        rloc_sb = consts.tile([128, GC], F32, name="rloc_sb")
        nc.sync.dma_start(rloc_sb[:], rloc_d[:])
        vals_sb = consts.tile([128, GC], F32, name="vals_sb")
        nc.sync.dma_start(vals_sb[:], vals_d[:])
        ident = consts.tile([128, 128], BF16, name="ident")
        make_identity(nc, ident[:])

        # ---- resident activations
        res = es.enter_context(tc.tile_pool(name="res", bufs=1))
        xh1_res = res.tile([128, PW, H], BF16, name="xh1_res")
        omlp1_res = res.tile([128, PW, H], BF16, name="omlp1_res")
        xh2_res = res.tile([128, PW, C], BF16, name="xh2_res")
        omlp2_res = res.tile([128, PW, C], BF16, name="omlp2_res")

        # ---- pools
        xt_pool = es.enter_context(tc.tile_pool(name="xt_pool", bufs=3))
        ps1_pool = es.enter_context(tc.tile_pool(name="ps1", bufs=2, space="PSUM"))
        psw_pool = es.enter_context(tc.tile_pool(name="psw", bufs=2, space="PSUM"))
        psT_pool = es.enter_context(tc.tile_pool(name="psT", bufs=2, space="PSUM"))
        ps2_pool = es.enter_context(tc.tile_pool(name="ps2", bufs=2, space="PSUM"))
        g_pool = es.enter_context(tc.tile_pool(name="g_pool", bufs=2))
        sval_pool = es.enter_context(tc.tile_pool(name="sval", bufs=6))
        wtmp_pool = es.enter_context(tc.tile_pool(name="wtmp", bufs=4))
        grp_pool = es.enter_context(tc.tile_pool(name="grp", bufs=2))
        sm_pool = es.enter_context(tc.tile_pool(name="sm", bufs=4))

        def emit_once():
            # ================= Phase A: layer-1 local matmuls =================
            for m in range(PW):
                xt_t = xt_pool.tile([128, FK, 128], BF16, name="xt_t")
                nc.sync.dma_start(xt_t[:], xt_d[m])
                ps = ps1_pool.tile([128, H3], F32, name="ps1_t")
                for kk in range(FK):
                    nc.tensor.matmul(out=ps[:], lhsT=xt_t[:, kk, :],
                                     rhs=w1_sb[:, kk, :],
                                     start=(kk == 0), stop=(kk == FK - 1))
                # [xl|xh] -> bf16 table + local copies
                t1w = wtmp_pool.tile([128, H2], BF16, name="t1w")
                nc.scalar.copy(t1w[:], ps[:, 0:H2])
                nc.sync.dma_start(t1_local[m * 128:(m + 1) * 128, :], t1w[:])
                nc.vector.tensor_copy(xh1_res[:, m, :], ps[:, H:H2])
                nc.scalar.activation(omlp1_res[:, m, :], ps[:, H2:H3], AF.Relu)

            # ================= Phase B: AllGather 1 =================
            nc.gpsimd.collective_compute(
                "AllGather", ALU.bypass,
                replica_groups=[list(range(NC))],
                ins=[t1_local[:].opt()],
                outs=[t1_full[:].opt()],
            )

            # ---- generic window-loop machinery (shared by both layers) ----
            def run_layer(layer, table, ew, xh_res, omlp_res, attl_sb, avec_sb,
                          sink):
                """layer: 1 or 2; table: DRAM tile [TBL, 2*ew]; ew: H or C.
                sink(w, wi, ws, h_tile): consume combined output per window."""
                ew2 = 2 * ew
                for gi, ws in enumerate(groups):
                    # gathers for this group
                    g_tiles = {}
                    split = cfg.get("SPLIT", 0)
                    for (hf, nch, base) in calls[gi]:
                        if nch == 0:
                            continue
                        gt = g_pool.tile([128, nch, ew2], BF16,
                                         name=f"g{layer}_{hf}", tag=f"gt{hf}")
                        src = table[0:HALF, :] if hf == 0 else table[HALF:TBL, :]
                        step = split if split > 0 else nch
                        nq = int(cfg.get("NQ", 1))
                        for qi, off in enumerate(range(0, nch, step)):
                            n = min(step, nch - off)
                            nc.gpsimd.dma_gather(
                                gt[:, off:off + n, :], src,
                                idx_sb[:, (base + off) * 8:(base + off + n) * 8],
                                n * 128, n * 128, ew2,
                                single_packet=bool(cfg.get("SP1", False)),
                                queue_num=qi % nq)
                        g_tiles[hf] = (gt, base)
                    olow_g = grp_pool.tile([128, len(ws), ew], BF16,
                                           name=f"olow{layer}", tag="olow")
                    ohigh_g = grp_pool.tile([128, len(ws), ew], BF16,
                                            name=f"ohigh{layer}", tag="ohigh")
                    feats_g = grp_pool.tile([128, len(ws), 3], F32,
                                            name=f"feats{layer}", tag="feats")
                    for wi, w in enumerate(ws):
                        ps = psw_pool.tile([128, ew2], F32, name="psw_t")
                        spans = []
                        for hf in (0, 1):
                            nch_w = int(gchunks[w, hf])
                            if nch_w == 0 or hf not in g_tiles:
                                continue
                            gt, base = g_tiles[hf]
                            cb = chunk_base[(w, hf)]
                            spans.append((gt, cb - base, cb, nch_w))
                        total = sum(s[3] for s in spans)
                        ci = 0
                        for (gt, loff, gcb, nch_w) in spans:
                            for c in range(nch_w):
                                sv = sval_pool.tile([128, 128], BF16, name="sv")
                                gcc = gcb + c
                                nc.vector.tensor_scalar(
                                    sv[:], colidx_sb[:],
                                    rloc_sb[:, gcc:gcc + 1],
                                    vals_sb[:, gcc:gcc + 1],
                                    ALU.is_equal, ALU.mult)
                                nc.tensor.matmul(out=ps[:], lhsT=sv[:],
                                                 rhs=gt[:, loff + c, :],
                                                 start=(ci == 0),
                                                 stop=(ci == total - 1))
                                ci += 1
                        if total == 0:
                            nc.vector.memset(ps[:], 0.0)
                        # o_low = relu(S_low)
                        nc.scalar.activation(olow_g[:, wi, :], ps[:, 0:ew], AF.Relu)
                        # o_high = relu(xh - S_high)
                        tmp = wtmp_pool.tile([128, ew], F32, name="ohtmp", tag="ohtmp")
                        nc.vector.tensor_tensor(out=tmp[:], in0=xh_res[:, w, :],
                                                in1=ps[:, ew:ew2], op=ALU.subtract)
                        nc.scalar.activation(ohigh_g[:, wi, :], tmp[:], AF.Relu)
                        # attention feats
                        pr = wtmp_pool.tile([128, 3, ew], F32, name="attn_pr",
                                            tag="attn_pr")
                        for j, src_t in enumerate((olow_g[:, wi, :],
                                                   ohigh_g[:, wi, :],
                                                   omlp_res[:, w, :])):
                            nc.vector.tensor_tensor(out=pr[:, j, :], in0=src_t,
                                                    in1=attl_sb[:, j, :],
                                                    op=ALU.mult)
                        nc.vector.tensor_reduce(feats_g[:, wi, :], pr[:],
                                                axis=AX.X, op=ALU.add)
                    # ---- attention (batched per group) ----
                    nw = len(ws)
                    sig = sm_pool.tile([128, nw, 3], F32, name="sig", tag="sig")
                    nc.scalar.activation(sig[:], feats_g[:], AF.Sigmoid)
                    zat = sm_pool.tile([128, nw, 3], F32, name="zat", tag="zat")
                    za = sm_pool.tile([128, nw], F32, name="za", tag="za")
                    zb = sm_pool.tile([128, nw], F32, name="zb", tag="zb")
                    for j in range(3):
                        nc.vector.tensor_scalar(za[:], sig[:, :, 0],
                                                avec_sb[:, 0 + j:1 + j], None,
                                                ALU.mult)
                        nc.vector.tensor_scalar(zb[:], sig[:, :, 1],
                                                avec_sb[:, 3 + j:4 + j], None,
                                                ALU.mult)
                        nc.vector.tensor_tensor(out=za[:], in0=za[:], in1=zb[:],
                                                op=ALU.add)
                        nc.vector.tensor_scalar(zb[:], sig[:, :, 2],
                                                avec_sb[:, 6 + j:7 + j], None,
                                                ALU.mult)
                        nc.vector.tensor_tensor(out=zat[:, :, j], in0=za[:],
                                                in1=zb[:], op=ALU.add)
                    mx = sm_pool.tile([128, nw], F32, name="mx", tag="mx")
                    nc.vector.tensor_reduce(mx[:], zat[:], axis=AX.X, op=ALU.max)
                    zs = sm_pool.tile([128, nw, 3], F32, name="zs", tag="zs")
                    nc.vector.tensor_tensor(
                        out=zs[:], in0=zat[:],
                        in1=mx[:].unsqueeze(2).to_broadcast([128, nw, 3]),
                        op=ALU.subtract)
                    ez = sm_pool.tile([128, nw, 3], F32, name="ez", tag="ez")
                    nc.scalar.activation(ez[:], zs[:], AF.Exp)
                    ssum = sm_pool.tile([128, nw], F32, name="ssum", tag="ssum")
                    nc.vector.tensor_reduce(ssum[:], ez[:], axis=AX.X, op=ALU.add)
                    rs = sm_pool.tile([128, nw], F32, name="rs", tag="rs")
                    nc.vector.reciprocal(rs[:], ssum[:])
                    nc.vector.tensor_scalar(rs[:], rs[:], 3.0, None, ALU.mult)
                    att = sm_pool.tile([128, nw, 3], F32, name="att", tag="att")
                    nc.vector.tensor_tensor(
                        out=att[:], in0=ez[:],
                        in1=rs[:].unsqueeze(2).to_broadcast([128, nw, 3]),
                        op=ALU.mult)
                    # ---- combine + sink per window ----
                    for wi, w in enumerate(ws):
                        ta = wtmp_pool.tile([128, ew], BF16, name="cmb_a", tag="cmb_a")
                        tb = wtmp_pool.tile([128, ew], BF16, name="cmb_b", tag="cmb_b")
                        hcomb = wtmp_pool.tile([128, ew], BF16, name="hcomb",
                                               tag="hcomb")
                        nc.vector.tensor_scalar(ta[:], olow_g[:, wi, :],
                                                att[:, wi, 0:1], None, ALU.mult)
                        nc.vector.tensor_scalar(tb[:], ohigh_g[:, wi, :],
                                                att[:, wi, 1:2], None, ALU.mult)
                        nc.vector.tensor_tensor(out=ta[:], in0=ta[:], in1=tb[:],
                                                op=ALU.add)
                        nc.vector.tensor_scalar(tb[:], omlp_res[:, w, :],
                                                att[:, wi, 2:3], None, ALU.mult)
                        nc.vector.tensor_tensor(out=hcomb[:], in0=ta[:], in1=tb[:],
                                                op=ALU.add)
                        sink(gi, ws, wi, w, hcomb)

            # ================= Phase C: layer-1 windows =================
            def sink1(gi, ws, wi, w, h_w):
                # transpose h_w, layer-2 local matmul
                psT = psT_pool.tile([128, H], BF16, name="psT_t")
                nc.tensor.transpose(psT[:], h_w[:], ident[:])
                hT = wtmp_pool.tile([128, H], BF16, name="hT", tag="hT")
                nc.vector.tensor_copy(hT[:], psT[:])
                ps2 = ps2_pool.tile([128, C3], F32, name="ps2_t")
                nc.tensor.matmul(out=ps2[:], lhsT=hT[:], rhs=w2_sb[:],
                                 start=True, stop=True)
                t2w = wtmp_pool.tile([128, C2], BF16, name="t2w", tag="t2w")
                nc.scalar.copy(t2w[:], ps2[:, 0:C2])
                nc.sync.dma_start(t2_local[w * 128:(w + 1) * 128, :], t2w[:])
                nc.vector.tensor_copy(xh2_res[:, w, :], ps2[:, C:C2])
                nc.scalar.activation(omlp2_res[:, w, :], ps2[:, C2:C3], AF.Relu)

            run_layer(1, t1_full, H, xh1_res, omlp1_res, attl1_sb, avec1_sb, sink1)

            # ================= Phase D: AllGather 2 =================
            nc.gpsimd.collective_compute(
                "AllGather", ALU.bypass,
                replica_groups=[list(range(NC))],
                ins=[t2_local[:].opt()],
                outs=[t2_full[:].opt()],
            )

            # ================= Phase E: layer-2 windows + log_softmax ========
            out_ap = out_d[:].rearrange("(w p) c -> p w c", p=128)
            h2_tiles = {}

            def sink2(gi, ws, wi, w, h_w):
                h2_g = h2_tiles[gi]
                nc.vector.tensor_copy(h2_g[:, wi, :], h_w[:])
                if wi == len(ws) - 1:
                    nw = len(ws)
                    mx2 = sm_pool.tile([128, nw], F32, name="mx2", tag="mx")
                    nc.vector.tensor_reduce(mx2[:], h2_g[:], axis=AX.X, op=ALU.max)
                    dd = grp_pool.tile([128, nw, C], F32, name="dd", tag="dd")
                    nc.vector.tensor_tensor(
                        out=dd[:], in0=h2_g[:],
                        in1=mx2[:].unsqueeze(2).to_broadcast([128, nw, C]),
                        op=ALU.subtract)
                    exd = grp_pool.tile([128, nw, C], F32, name="exd", tag="exd")
                    nc.scalar.activation(exd[:], dd[:], AF.Exp)
                    s2 = sm_pool.tile([128, nw], F32, name="s2", tag="ssum")
                    nc.vector.tensor_reduce(s2[:], exd[:], axis=AX.X, op=ALU.add)
                    ln2 = sm_pool.tile([128, nw], F32, name="ln2", tag="rs")
                    nc.scalar.activation(ln2[:], s2[:], AF.Ln)
                    ot = grp_pool.tile([128, nw, C], F32, name="ot", tag="ot")
                    nc.vector.tensor_tensor(
                        out=ot[:], in0=dd[:],
                        in1=ln2[:].unsqueeze(2).to_broadcast([128, nw, C]),
                        op=ALU.subtract)
                    nc.sync.dma_start(out_ap[:, ws[0]:ws[0] + nw, :], ot[:])

            def run_layer2():
                def presink(gi, ws, wi, w, h_w):
                    if gi not in h2_tiles:
                        h2_tiles[gi] = grp_pool.tile([128, len(ws), C], F32,
                                                     name="h2g", tag="h2g")
                    sink2(gi, ws, wi, w, h_w)
                run_layer(2, t2_full, C, xh2_res, omlp2_res, attl2_sb, avec2_sb,
                          presink)

            run_layer2()

        for _rep in range(repeat):
            emit_once()
        es.close()

    nc.compile()
    return nc


# --------------------------------------------------------------------------
# Runner (cached compiled program + jitted PJRT executable)
# --------------------------------------------------------------------------

_CACHE = {}


class _Runner:
    def __init__(self, plan, cfg):
        self.cfg = cfg
        self.plan = plan
        self.nc = build_program(plan, cfg)
        self._fn = None

    def _build_fn(self):
        import jax
        from jax.sharding import Mesh, PartitionSpec
        from jax.experimental.shard_map import shard_map
        from concourse import bass2jax

        nc = self.nc
        NC = self.cfg["NC"]
        bass2jax.install_neuronx_cc_hook()
        partition_name = (nc.partition_id_tensor.name
                          if nc.partition_id_tensor else None)
        in_names, out_names, out_avals, zero_outs = [], [], [], []
        for alloc in nc.m.functions[0].allocations:
            if not isinstance(alloc, mybir.MemoryLocationSet):
                continue
            name = alloc.memorylocations[0].name
            if alloc.kind == "ExternalInput":
                if name != partition_name:
                    in_names.append(name)
            elif alloc.kind == "ExternalOutput":
                shape = tuple(alloc.tensor_shape)
                dtype = mybir.dt.np(alloc.dtype)
                out_avals.append(jax.core.ShapedArray(shape, dtype))
                out_names.append(name)
                zero_outs.append(np.zeros(shape, dtype))
        n_params = len(in_names)
        bind_in_names = list(in_names) + list(out_names)
        if partition_name is not None:
            bind_in_names.append(partition_name)

        def _body(*args):
            operands = list(args)
            if partition_name is not None:
                operands.append(bass2jax.partition_id_tensor())
            outs = bass2jax._bass_exec_p.bind(
                *operands,
                out_avals=tuple(out_avals),
                in_names=tuple(bind_in_names),
                out_names=tuple(out_names),
                lowering_input_output_aliases=(),
                sim_require_finite=True,
                sim_require_nnan=True,
                nc=nc,
            )
            return tuple(outs)

        devices = jax.devices()[:NC]
        mesh = Mesh(np.asarray(devices), ("core",))
        n_outs = len(out_names)
        in_specs = (PartitionSpec("core"),) * (n_params + n_outs)
        out_specs = (PartitionSpec("core"),) * n_outs
        fn = jax.jit(
            shard_map(_body, mesh=mesh, in_specs=in_specs,
                      out_specs=out_specs, check_rep=False),
            keep_unused=True)
        self._fn = fn
        self._in_names = in_names
        self._out_names = out_names
        self._out_avals = out_avals
        self._zero_outs = zero_outs

    def prepare_args(self, in_maps):
        import jax
        NC = self.cfg["NC"]
        per_core = [[np.asarray(m[name]) for name in self._in_names]
                    for m in in_maps]
        concat_in = [np.concatenate([per_core[c][i] for c in range(NC)], axis=0)
                     for i in range(len(self._in_names))]
        concat_zeros = [np.zeros((NC * z.shape[0], *z.shape[1:]), z.dtype)
                        for z in self._zero_outs]
        return [jax.device_put(a) for a in concat_in + concat_zeros]

    def time_ns(self, in_maps, r_hi=6, reps=40):
        """Per-execution device time, measured by differencing wall times of
        this NEFF vs a variant whose body repeats the whole kernel r_hi times
        (fixed RPC + input-staging costs cancel in the difference)."""
        import time
        import jax

        if self._fn is None:
            self._build_fn()
        if not hasattr(self, "_fn_hi") or self._fn_hi is None:
            rh = _Runner.__new__(_Runner)
            rh.cfg = self.cfg
            rh.plan = self.plan
            rh.nc = build_program(self.plan, self.cfg, repeat=r_hi)
            rh._fn = None
            rh._build_fn()
            self._fn_hi = rh._fn
            self._rh = rh
            self._r_hi = r_hi
        a1 = self.prepare_args(in_maps)
        ah = self._rh.prepare_args(in_maps)
        jax.block_until_ready(self._fn(*a1))
        jax.block_until_ready(self._fn_hi(*ah))
        t1s, ths = [], []
        for _ in range(reps):
            t0 = time.perf_counter()
            jax.block_until_ready(self._fn(*a1))
            t1s.append(time.perf_counter() - t0)
            t0 = time.perf_counter()
            jax.block_until_ready(self._fn_hi(*ah))
            ths.append(time.perf_counter() - t0)
        t1s.sort()
        ths.sort()
        i = max(1, reps // 10)
        return (ths[i] - t1s[i]) / (self._r_hi - 1) * 1e9

    def run(self, in_maps):
        import jax
        if self._fn is None:
            self._build_fn()
        args = self.prepare_args(in_maps)
        outs = self._fn(*args)
        jax.block_until_ready(outs)
        NC = self.cfg["NC"]
        res = []
        for c in range(NC):
            m = {}
            for i, name in enumerate(self._out_names):
                m[name] = np.asarray(outs[i]).reshape(
                    NC, *self._out_avals[i].shape)[c]
            res.append(m)
        return res


def get_runner(inputs, cfg=None):
    cfg = dict(DEFAULT_CFG if cfg is None else cfg)
    plan = make_plan(np.asarray(inputs["edge_row"]).astype(np.int64),
                     np.asarray(inputs["edge_col"]).astype(np.int64), cfg)
    key = plan_key(plan, cfg)
    if key not in _CACHE:
        _CACHE[key] = _Runner(plan, cfg)
    return _CACHE[key], plan


def kernel(**inputs) -> np.ndarray:
    cfg = dict(DEFAULT_CFG)
    runner, plan = get_runner(inputs, cfg)
    in_maps = pack_inputs(inputs, runner.plan, cfg)
    res = runner.run(in_maps)
    NSH = runner.plan["d"]["NSH"]
    out = np.concatenate([res[k]["out"][:NSH] for k in range(cfg["NC"])],
                         axis=0)
    return out[:cfg["N"]].astype(np.float32)


_CALIB = {}


def _calib_runner(n_cores):
    """Tiny NEFF used to measure the per-execution dispatch overhead."""
    if n_cores in _CALIB:
        return _CALIB[n_cores]
    import jax
    import concourse.tile as tile_mod

    nc = bacc.Bacc(None, target_bir_lowering=False, num_devices=n_cores)
    inp = nc.declare_dram_parameter("inp", [128, 64], F32, isOutput=False)
    out = nc.declare_dram_parameter("out", [128, 64], F32, isOutput=True)
    with tile_mod.TileContext(nc) as tc:
        with tc.tile_pool(name="sbuf", bufs=2) as sbuf:
            t = sbuf.tile([128, 64], F32, name="t")
            nc.sync.dma_start(t[:], inp[:])
            nc.vector.tensor_scalar(t[:], t[:], 2.0, None, ALU.mult)
            nc.sync.dma_start(out[:], t[:])
    nc.compile()

    class _Tmp:
        pass

    r = _Runner.__new__(_Runner)
    r.cfg = dict(NC=n_cores)
    r.nc = nc
    r._fn = None
    r._build_fn()
    x = np.zeros((128, 64), np.float32)
    args = r.prepare_args([{"inp": x} for _ in range(n_cores)])
    _CALIB[n_cores] = (r._fn, args)
    return _CALIB[n_cores]



# revision 16
# speedup vs baseline: 1.1542x; 1.1542x over previous
"""ACM-GCN (2-layer) distributed Bass kernel for 8 TRN2 NeuronCores.

Strategy:
  - Shard nodes (rows of x / segment ids) across 8 cores: core k owns global
    rows [k*6250, (k+1)*6250), padded to 6272 = 49*128 per core.
  - Layer matmuls (x @ W_*) are local per core (lhsT = pre-transposed x tiles
    shipped from host in bf16).
  - SpMM (adj_low @ xl etc): each core owns the edges whose DESTINATION row
    lives on it. The [xl|xh] activations are AllGathered into a per-core HBM
    table; per 128-row destination window the kernel dma_gathers the source
    rows (bf16, 512B rows) and segment-sums them with one-hot val matmuls
    accumulated in PSUM (lhsT = one-hot(val) built on DVE with a fused
    tensor_scalar is_equal*mult).
  - Attention + combine + log_softmax are node-parallel (local).

All metadata (edge->window assignment, gather indices, one-hot row ids and
vals) is packed host-side in numpy; the Bass graph is static with
data-dependent chunk capacities shared across all 8 cores (max over cores).
"""

import math

import numpy as np
import ml_dtypes

import concourse.bass as bass
import concourse.mybir as mybir
import concourse.tile as tile
from concourse import bacc
from concourse.masks import make_identity

F32 = mybir.dt.float32
BF16 = mybir.dt.bfloat16
I16 = mybir.dt.int16
BF = ml_dtypes.bfloat16
AF = mybir.ActivationFunctionType
ALU = mybir.AluOpType
AX = mybir.AxisListType

DEFAULT_CFG = dict(N=50000, F=512, H=128, C=64, NC=8, GROUP=4, SPLIT=0, NQ=4)


# --------------------------------------------------------------------------
# Host-side planning / packing
# --------------------------------------------------------------------------

def derive(cfg):
    N, F, H, C, NC = cfg["N"], cfg["F"], cfg["H"], cfg["C"], cfg["NC"]
    assert N % NC == 0
    NSH = N // NC
    PW = (NSH + 127) // 128
    NPAD = PW * 128
    TBL = NC * NPAD
    HALF = TBL // 2
    assert HALF <= 32768, "int16 gather index limit"
    assert F % 128 == 0
    FK = F // 128
    return dict(NSH=NSH, PW=PW, NPAD=NPAD, TBL=TBL, HALF=HALF, FK=FK)


def make_plan(edge_row, edge_col, cfg):
    """Static shape plan shared by all cores: chunk counts per (window, half),
    grouping, chunk bases."""
    d = derive(cfg)
    NC, GROUP = cfg["NC"], cfg["GROUP"]
    NSH, PW, NPAD, HALF = d["NSH"], d["PW"], d["NPAD"], d["HALF"]

    core = edge_row // NSH
    dr = edge_row - core * NSH
    w = dr // 128
    tcol = (edge_col // NSH) * NPAD + (edge_col % NSH)
    half = (tcol >= HALF).astype(np.int64)
    key = (core.astype(np.int64) * PW + w) * 2 + half
    counts = np.bincount(key, minlength=NC * PW * 2).reshape(NC, PW, 2)
    gchunks = (counts.max(axis=0) + 127) // 128  # [PW, 2]

    groups = [list(range(g, min(g + GROUP, PW))) for g in range(0, PW, GROUP)]
    chunk_base = {}
    gc = 0
    calls = []  # per group: [(half, total_chunks, base_chunk)]
    for ws in groups:
        gcalls = []
        for hf in (0, 1):
            base = gc
            for wi in ws:
                chunk_base[(wi, hf)] = gc
                gc += int(gchunks[wi, hf])
            gcalls.append((hf, gc - base, base))
        calls.append(gcalls)
    return dict(d=d, gchunks=gchunks, groups=groups, chunk_base=chunk_base,
                calls=calls, GC=gc)


def plan_key(plan, cfg):
    return (tuple(sorted(cfg.items())),
            tuple(map(int, plan["gchunks"].flatten())))


def pack_inputs(inputs, plan, cfg):
    """Build per-core in_maps (numpy) for the bass program."""
    d = plan["d"]
    N, F, H, C, NC = cfg["N"], cfg["F"], cfg["H"], cfg["C"], cfg["NC"]
    NSH, PW, NPAD, HALF, FK = d["NSH"], d["PW"], d["NPAD"], d["HALF"], d["FK"]
    GC = plan["GC"]
    chunk_base = plan["chunk_base"]

    x = np.asarray(inputs["x"], np.float32)
    er = np.asarray(inputs["edge_row"]).astype(np.int64)
    ec = np.asarray(inputs["edge_col"]).astype(np.int64)
    ev = np.asarray(inputs["edge_val"], np.float32)

    w1 = np.concatenate([np.asarray(inputs["weight_low"], np.float32),
                         np.asarray(inputs["weight_high"], np.float32),
                         np.asarray(inputs["weight_mlp"], np.float32)], axis=1)  # [F, 3H]
    w2 = np.concatenate([np.asarray(inputs["weight_low2"], np.float32),
                         np.asarray(inputs["weight_high2"], np.float32),
                         np.asarray(inputs["weight_mlp2"], np.float32)], axis=1)  # [H, 3C]
    # w1 sbuf layout [128, FK, 3H]: [p, kk, n] = w1[kk*128+p, n]
    w1_sb = np.ascontiguousarray(
        w1.reshape(FK, 128, 3 * H).transpose(1, 0, 2)).astype(BF)
    w2_sb = w2.astype(BF)  # [H=128, 3C]

    attl1 = np.stack([np.asarray(inputs["att_vec_low"], np.float32)[:, 0],
                      np.asarray(inputs["att_vec_high"], np.float32)[:, 0],
                      np.asarray(inputs["att_vec_mlp"], np.float32)[:, 0]])  # [3, H]
    attl2 = np.stack([np.asarray(inputs["att_vec_low2"], np.float32)[:, 0],
                      np.asarray(inputs["att_vec_high2"], np.float32)[:, 0],
                      np.asarray(inputs["att_vec_mlp2"], np.float32)[:, 0]])  # [3, C]
    attl1_sb = np.broadcast_to(attl1[None], (128, 3, H)).astype(BF).copy()
    attl2_sb = np.broadcast_to(attl2[None], (128, 3, C)).astype(BF).copy()
    avec1 = (np.asarray(inputs["att_vec"], np.float32) / 3.0).reshape(9)
    avec2 = (np.asarray(inputs["att_vec2"], np.float32) / 3.0).reshape(9)
    avec1_sb = np.broadcast_to(avec1[None], (128, 9)).astype(np.float32).copy()
    avec2_sb = np.broadcast_to(avec2[None], (128, 9)).astype(np.float32).copy()
    colidx = np.broadcast_to(np.arange(128, dtype=np.float32)[None], (128, 128)
                             ).astype(BF).copy()

    # edge metadata
    core = er // NSH
    dr = er - core * NSH
    w = dr // 128
    rl = (dr % 128).astype(np.float32)
    tcol = (ec // NSH) * NPAD + (ec % NSH)
    half = (tcol >= HALF).astype(np.int64)
    idxv = (tcol - half * HALF).astype(np.int64)

    in_maps = []
    for k in range(NC):
        sel = core == k
        kw, khalf, kidx, krl, kval = w[sel], half[sel], idxv[sel], rl[sel], ev[sel]
        # order edges by (w, half)
        order = np.lexsort((khalf, kw))
        kw, khalf, kidx, krl, kval = (a[order] for a in (kw, khalf, kidx, krl, kval))
        seg_key = kw * 2 + khalf
        seg_counts = np.bincount(seg_key, minlength=PW * 2).reshape(PW, 2)
        # global slot for each edge: chunk_base[(w,half)]*128 + position-in-segment
        # position-in-segment:
        starts = np.zeros(PW * 2, np.int64)
        np.cumsum(seg_counts.flatten()[:-1], out=starts[1:])
        pos_in_seg = np.arange(len(kw)) - starts[seg_key]
        cb = np.array([[chunk_base[(wi, hf)] for hf in (0, 1)] for wi in range(PW)],
                      np.int64)
        slot = cb[kw, khalf] * 128 + (pos_in_seg // 128) * 128 + pos_in_seg % 128
        # (pos_in_seg fits within gchunks[w,half]*128 by construction)

        idx_flat = np.zeros(GC * 128, np.int16)
        rl_arr = np.full((128, GC), -1.0, np.float32)
        val_arr = np.zeros((128, GC), np.float32)
        idx_flat[slot] = kidx.astype(np.int16)
        p = slot % 128
        c = slot // 128
        rl_arr[p, c] = krl
        val_arr[p, c] = kval
        rl_arr = rl_arr.astype(BF)
        val_arr = val_arr.astype(BF)
        # idxs layout for dma_gather: [j%16, j//16] replicated over 8 groups of
        # 16 partitions
        idx16 = np.tile(idx_flat.reshape(-1, 16).T, (8, 1))  # [128, GC*8]

        # pre-transposed x tiles: [PW, 128, FK, 128]:
        # xt[m, p, kk, j] = x[k*NSH + m*128 + j, kk*128 + p]
        xk = np.zeros((NPAD, F), np.float32)
        xk[:NSH] = x[k * NSH:(k + 1) * NSH]
        xt = np.ascontiguousarray(
            xk.reshape(PW, 128, FK, 128).transpose(0, 3, 2, 1)).astype(BF)
        # check: xt[m, pfeat, kk, jnode] = xk[m*128 + jnode, kk*128 + pfeat]

        in_maps.append({
            "xt": xt,
            "w1s": w1_sb, "w2s": w2_sb,
            "attl1": attl1_sb, "attl2": attl2_sb,
            "avec1": avec1_sb, "avec2": avec2_sb,
            "colidx": colidx,
            "idx16": idx16,
            "rloc": rl_arr,
            "vals": val_arr,
        })
    return in_maps


# --------------------------------------------------------------------------
# Bass program
# --------------------------------------------------------------------------

def build_program(plan, cfg, repeat=1):
    d = plan["d"]
    N, F, H, C, NC = cfg["N"], cfg["F"], cfg["H"], cfg["C"], cfg["NC"]
    NSH, PW, NPAD, TBL, HALF, FK = (d["NSH"], d["PW"], d["NPAD"], d["TBL"],
                                    d["HALF"], d["FK"])
    GC = plan["GC"]
    gchunks = plan["gchunks"]
    groups = plan["groups"]
    chunk_base = plan["chunk_base"]
    calls = plan["calls"]
    H2, C2, C3 = 2 * H, 2 * C, 3 * C
    H3 = 3 * H

    nc = bacc.Bacc(None, target_bir_lowering=False, num_devices=NC,
                   num_swdge_queues=int(cfg.get("NQ", 1)))

    xt_d = nc.declare_dram_parameter("xt", [PW, 128, FK, 128], BF16, isOutput=False)
    w1_d = nc.declare_dram_parameter("w1s", [128, FK, H3], BF16, isOutput=False)
    w2_d = nc.declare_dram_parameter("w2s", [H, C3], BF16, isOutput=False)
    attl1_d = nc.declare_dram_parameter("attl1", [128, 3, H], BF16, isOutput=False)
    attl2_d = nc.declare_dram_parameter("attl2", [128, 3, C], BF16, isOutput=False)
    avec1_d = nc.declare_dram_parameter("avec1", [128, 9], F32, isOutput=False)
    avec2_d = nc.declare_dram_parameter("avec2", [128, 9], F32, isOutput=False)
    colidx_d = nc.declare_dram_parameter("colidx", [128, 128], BF16, isOutput=False)
    idx16_d = nc.declare_dram_parameter("idx16", [128, GC * 8], I16, isOutput=False)
    rloc_d = nc.declare_dram_parameter("rloc", [128, GC], BF16, isOutput=False)
    vals_d = nc.declare_dram_parameter("vals", [128, GC], BF16, isOutput=False)
    out_d = nc.declare_dram_parameter("out", [NPAD, C], F32, isOutput=True)

    with tile.TileContext(nc) as tc:
        # ---- DRAM internals
        from contextlib import ExitStack
        es = ExitStack()
        dram_pool = es.enter_context(
            tc.tile_pool(name="dram_pool", bufs=1, space="DRAM"))

        # ---- resident constants / metadata
        consts = es.enter_context(tc.tile_pool(name="consts", bufs=1))
        w1_sb = consts.tile([128, FK, H3], BF16, name="w1_sb")
        nc.sync.dma_start(w1_sb[:], w1_d[:])
        w2_sb = consts.tile([H, C3], BF16, name="w2_sb")
        nc.sync.dma_start(w2_sb[:], w2_d[:])
        attl1_sb = consts.tile([128, 3, H], BF16, name="attl1_sb")
        nc.sync.dma_start(attl1_sb[:], attl1_d[:])
        attl2_sb = consts.tile([128, 3, C], BF16, name="attl2_sb")
        nc.sync.dma_start(attl2_sb[:], attl2_d[:])
        avec1_sb = consts.tile([128, 9], F32, name="avec1_sb")
        nc.sync.dma_start(avec1_sb[:], avec1_d[:])
        avec2_sb = consts.tile([128, 9], F32, name="avec2_sb")
        nc.sync.dma_start(avec2_sb[:], avec2_d[:])
        colidx_sb = consts.tile([128, 128], BF16, name="colidx_sb")
        nc.sync.dma_start(colidx_sb[:], colidx_d[:])
        idx_sb = consts.tile([128, GC * 8], I16, name="idx_sb")
        nc.sync.dma_start(idx_sb[:], idx16_d[:])
        rloc_sb = consts.tile([128, GC], BF16, name="rloc_sb")
        nc.sync.dma_start(rloc_sb[:], rloc_d[:])
        vals_sb = consts.tile([128, GC], BF16, name="vals_sb")
        nc.sync.dma_start(vals_sb[:], vals_d[:])
        ident = consts.tile([128, 128], BF16, name="ident")
        make_identity(nc, ident[:])

        # ---- resident activations
        res = es.enter_context(tc.tile_pool(name="res", bufs=1))
        xh1_res = res.tile([128, PW, H], BF16, name="xh1_res")
        omlp1_res = res.tile([128, PW, H], BF16, name="omlp1_res")
        xh2_res = res.tile([128, PW, C], BF16, name="xh2_res")
        omlp2_res = res.tile([128, PW, C], BF16, name="omlp2_res")

        # ---- pools
        xt_pool = es.enter_context(tc.tile_pool(name="xt_pool", bufs=3))
        ps1_pool = es.enter_context(tc.tile_pool(name="ps1", bufs=2, space="PSUM"))
        psw_pool = es.enter_context(tc.tile_pool(name="psw", bufs=2, space="PSUM"))
        psT_pool = es.enter_context(tc.tile_pool(name="psT", bufs=2, space="PSUM"))
        ps2_pool = es.enter_context(tc.tile_pool(name="ps2", bufs=2, space="PSUM"))
        g_pool = es.enter_context(tc.tile_pool(name="g_pool", bufs=2))
        sval_pool = es.enter_context(tc.tile_pool(name="sval", bufs=2))
        wtmp_pool = es.enter_context(tc.tile_pool(name="wtmp", bufs=3))
        grp_pool = es.enter_context(tc.tile_pool(name="grp", bufs=2))
        sm_pool = es.enter_context(tc.tile_pool(name="sm", bufs=4))

        rep_ctr = [0]

        def emit_once():
            rep = rep_ctr[0]
            rep_ctr[0] += 1
            qrr = [0]  # round-robin SWDGE queue counter for gathers
            t1_local = dram_pool.tile([NPAD, H2], BF16, name="t1_local",
                                      tag=f"t1l{rep}")
            t1_full = dram_pool.tile([TBL, H2], BF16, name="t1_full",
                                     tag=f"t1f{rep}", addr_space="Shared")
            t2_local = dram_pool.tile([NPAD, C2], BF16, name="t2_local",
                                      tag=f"t2l{rep}")
            t2_full = dram_pool.tile([TBL, C2], BF16, name="t2_full",
                                     tag=f"t2f{rep}", addr_space="Shared")
            # ================= Phase A: layer-1 local matmuls =================
            for m in range(PW):
                xt_t = xt_pool.tile([128, FK, 128], BF16, name="xt_t")
                nc.sync.dma_start(xt_t[:], xt_d[m])
                ps = ps1_pool.tile([128, H3], F32, name="ps1_t")
                for kk in range(FK):
                    nc.tensor.matmul(out=ps[:], lhsT=xt_t[:, kk, :],
                                     rhs=w1_sb[:, kk, :],
                                     start=(kk == 0), stop=(kk == FK - 1))
                # [xl|xh] -> bf16 table + local copies
                t1w = wtmp_pool.tile([128, H2], BF16, name="t1w")
                nc.scalar.copy(t1w[:], ps[:, 0:H2])
                nc.sync.dma_start(t1_local[m * 128:(m + 1) * 128, :], t1w[:])
                nc.vector.tensor_copy(xh1_res[:, m, :], ps[:, H:H2])
                nc.scalar.activation(omlp1_res[:, m, :], ps[:, H2:H3], AF.Relu)

            # ================= Phase B: AllGather 1 =================
            nc.gpsimd.collective_compute(
                "AllGather", ALU.bypass,
                replica_groups=[list(range(NC))],
                ins=[t1_local[:].opt()],
                outs=[t1_full[:].opt()],
            )

            # ---- generic window-loop machinery (shared by both layers) ----
            def run_layer(layer, table, ew, xh_res, omlp_res, attl_sb, avec_sb,
                          sink):
                """layer: 1 or 2; table: DRAM tile [TBL, 2*ew]; ew: H or C.
                sink(w, wi, ws, h_tile): consume combined output per window."""
                ew2 = 2 * ew
                nq = int(cfg.get("NQ", 1))
                for gi, ws in enumerate(groups):
                    nw = len(ws)
                    # gathers for this group (one call per half, rr queues)
                    g_tiles = {}
                    for (hf, nch, base) in calls[gi]:
                        if nch == 0:
                            continue
                        gt = g_pool.tile([128, nch, ew2], BF16,
                                         name=f"g{layer}_{hf}", tag=f"gt{hf}")
                        src = table[0:HALF, :] if hf == 0 else table[HALF:TBL, :]
                        nc.gpsimd.dma_gather(
                            gt[:, :, :], src,
                            idx_sb[:, base * 8:(base + nch) * 8],
                            nch * 128, nch * 128, ew2,
                            single_packet=bool(cfg.get("SP1", False)),
                            queue_num=qrr[0] % nq)
                        qrr[0] += 1
                        g_tiles[hf] = (gt, base)
                    # batched one-hot generation for the whole group's chunks
                    gbase = calls[gi][0][2]
                    gtot = sum(nch for (_hf, nch, _b) in calls[gi])
                    sv_g = sval_pool.tile([128, gtot, 128], BF16, name="sv_g",
                                          tag="sv_g")
                    nc.vector.tensor_tensor(
                        out=sv_g[:],
                        in0=colidx_sb[:].unsqueeze(1).to_broadcast(
                            [128, gtot, 128]),
                        in1=rloc_sb[:, gbase:gbase + gtot].unsqueeze(2)
                            .to_broadcast([128, gtot, 128]),
                        op=ALU.is_equal)
                    nc.vector.tensor_tensor(
                        out=sv_g[:], in0=sv_g[:],
                        in1=vals_sb[:, gbase:gbase + gtot].unsqueeze(2)
                            .to_broadcast([128, gtot, 128]),
                        op=ALU.mult)
                    olow_g = grp_pool.tile([128, nw, ew], BF16,
                                           name=f"olow{layer}", tag="olow")
                    ohigh_g = grp_pool.tile([128, nw, ew], BF16,
                                            name=f"ohigh{layer}", tag="ohigh")
                    feats_g = grp_pool.tile([128, nw, 3], F32,
                                            name=f"feats{layer}", tag="feats")
                    for wi, w in enumerate(ws):
                        ps = psw_pool.tile([128, ew2], F32, name="psw_t")
                        spans = []
                        for hf in (0, 1):
                            nch_w = int(gchunks[w, hf])
                            if nch_w == 0 or hf not in g_tiles:
                                continue
                            gt, base = g_tiles[hf]
                            cb = chunk_base[(w, hf)]
                            spans.append((gt, cb - base, cb, nch_w))
                        total = sum(s[3] for s in spans)
                        ci = 0
                        for (gt, loff, gcb, nch_w) in spans:
                            for c in range(nch_w):
                                gcc = gcb + c
                                nc.tensor.matmul(out=ps[:],
                                                 lhsT=sv_g[:, gcc - gbase, :],
                                                 rhs=gt[:, loff + c, :],
                                                 start=(ci == 0),
                                                 stop=(ci == total - 1))
                                ci += 1
                        if total == 0:
                            nc.vector.memset(ps[:], 0.0)
                        # o_low = relu(S_low)
                        nc.scalar.activation(olow_g[:, wi, :], ps[:, 0:ew], AF.Relu)
                        # o_high = relu(xh - S_high)
                        tmp = wtmp_pool.tile([128, ew], F32, name="ohtmp", tag="ohtmp")
                        nc.vector.tensor_tensor(out=tmp[:], in0=xh_res[:, w, :],
                                                in1=ps[:, ew:ew2], op=ALU.subtract)
                        nc.scalar.activation(ohigh_g[:, wi, :], tmp[:], AF.Relu)
                    # attention feats (batched per group)
                    pr = wtmp_pool.tile([128, nw, 3, ew], BF16, name="attn_pr",
                                        tag="attn_pr")
                    for j, src_t in enumerate((olow_g[:], ohigh_g[:],
                                               omlp_res[:, ws[0]:ws[0] + nw, :])):
                        nc.vector.tensor_tensor(
                            out=pr[:, :, j, :], in0=src_t,
                            in1=attl_sb[:, j, :].unsqueeze(1)
                                .to_broadcast([128, nw, ew]),
                            op=ALU.mult)
                    nc.vector.tensor_reduce(feats_g[:], pr[:],
                                            axis=AX.X, op=ALU.add)
                    # ---- attention (batched per group) ----
                    sig = sm_pool.tile([128, nw, 3], F32, name="sig", tag="sig")
                    nc.scalar.activation(sig[:], feats_g[:], AF.Sigmoid)
                    zat = sm_pool.tile([128, nw, 3], F32, name="zat", tag="zat")
                    za = sm_pool.tile([128, nw], F32, name="za", tag="za")
                    zb = sm_pool.tile([128, nw], F32, name="zb", tag="zb")
                    for j in range(3):
                        nc.vector.tensor_scalar(za[:], sig[:, :, 0],
                                                avec_sb[:, 0 + j:1 + j], None,
                                                ALU.mult)
                        nc.vector.tensor_scalar(zb[:], sig[:, :, 1],
                                                avec_sb[:, 3 + j:4 + j], None,
                                                ALU.mult)
                        nc.vector.tensor_tensor(out=za[:], in0=za[:], in1=zb[:],
                                                op=ALU.add)
                        nc.vector.tensor_scalar(zb[:], sig[:, :, 2],
                                                avec_sb[:, 6 + j:7 + j], None,
                                                ALU.mult)
                        nc.vector.tensor_tensor(out=zat[:, :, j], in0=za[:],
                                                in1=zb[:], op=ALU.add)
                    mx = sm_pool.tile([128, nw], F32, name="mx", tag="mx")
                    nc.vector.tensor_reduce(mx[:], zat[:], axis=AX.X, op=ALU.max)
                    zs = sm_pool.tile([128, nw, 3], F32, name="zs", tag="zs")
                    nc.vector.tensor_tensor(
                        out=zs[:], in0=zat[:],
                        in1=mx[:].unsqueeze(2).to_broadcast([128, nw, 3]),
                        op=ALU.subtract)
                    ez = sm_pool.tile([128, nw, 3], F32, name="ez", tag="ez")
                    nc.scalar.activation(ez[:], zs[:], AF.Exp)
                    ssum = sm_pool.tile([128, nw], F32, name="ssum", tag="ssum")
                    nc.vector.tensor_reduce(ssum[:], ez[:], axis=AX.X, op=ALU.add)
                    rs = sm_pool.tile([128, nw], F32, name="rs", tag="rs")
                    nc.vector.reciprocal(rs[:], ssum[:])
                    nc.vector.tensor_scalar(rs[:], rs[:], 3.0, None, ALU.mult)
                    att = sm_pool.tile([128, nw, 3], F32, name="att", tag="att")
                    nc.vector.tensor_tensor(
                        out=att[:], in0=ez[:],
                        in1=rs[:].unsqueeze(2).to_broadcast([128, nw, 3]),
                        op=ALU.mult)
                    # ---- combine (batched per group) + sink per window ----
                    ta = wtmp_pool.tile([128, nw, ew], BF16, name="cmb_a",
                                        tag="cmb_a")
                    tb = wtmp_pool.tile([128, nw, ew], BF16, name="cmb_b",
                                        tag="cmb_b")
                    hcomb = wtmp_pool.tile([128, nw, ew], BF16, name="hcomb",
                                           tag="hcomb")
                    nc.vector.tensor_tensor(
                        out=ta[:], in0=olow_g[:],
                        in1=att[:, :, 0].unsqueeze(2).to_broadcast([128, nw, ew]),
                        op=ALU.mult)
                    nc.vector.tensor_tensor(
                        out=tb[:], in0=ohigh_g[:],
                        in1=att[:, :, 1].unsqueeze(2).to_broadcast([128, nw, ew]),
                        op=ALU.mult)
                    nc.vector.tensor_tensor(out=ta[:], in0=ta[:], in1=tb[:],
                                            op=ALU.add)
                    nc.vector.tensor_tensor(
                        out=tb[:], in0=omlp_res[:, ws[0]:ws[0] + nw, :],
                        in1=att[:, :, 2].unsqueeze(2).to_broadcast([128, nw, ew]),
                        op=ALU.mult)
                    nc.vector.tensor_tensor(out=hcomb[:], in0=ta[:], in1=tb[:],
                                            op=ALU.add)
                    sink(gi, ws, hcomb)

            # ================= Phase C: layer-1 windows =================
            def sink1(gi, ws, h_g):
                # transpose h per window, layer-2 local matmul
                for wi, w in enumerate(ws):
                    psT = psT_pool.tile([128, H], BF16, name="psT_t")
                    nc.tensor.transpose(psT[:], h_g[:, wi, :], ident[:])
                    hT = wtmp_pool.tile([128, H], BF16, name="hT", tag="hT")
                    nc.vector.tensor_copy(hT[:], psT[:])
                    ps2 = ps2_pool.tile([128, C3], F32, name="ps2_t")
                    nc.tensor.matmul(out=ps2[:], lhsT=hT[:], rhs=w2_sb[:],
                                     start=True, stop=True)
                    t2w = wtmp_pool.tile([128, C2], BF16, name="t2w", tag="t2w")
                    nc.scalar.copy(t2w[:], ps2[:, 0:C2])
                    nc.sync.dma_start(t2_local[w * 128:(w + 1) * 128, :], t2w[:])
                    nc.vector.tensor_copy(xh2_res[:, w, :], ps2[:, C:C2])
                    nc.scalar.activation(omlp2_res[:, w, :], ps2[:, C2:C3],
                                         AF.Relu)

            run_layer(1, t1_full, H, xh1_res, omlp1_res, attl1_sb, avec1_sb, sink1)

            # ================= Phase D: AllGather 2 =================
            nc.gpsimd.collective_compute(
                "AllGather", ALU.bypass,
                replica_groups=[list(range(NC))],
                ins=[t2_local[:].opt()],
                outs=[t2_full[:].opt()],
            )

            # ================= Phase E: layer-2 windows + log_softmax ========
            out_ap = out_d[:].rearrange("(w p) c -> p w c", p=128)

            def sink2(gi, ws, h_g):
                nw = len(ws)
                mx2 = sm_pool.tile([128, nw], F32, name="mx2", tag="mx")
                nc.vector.tensor_reduce(mx2[:], h_g[:], axis=AX.X, op=ALU.max)
                dd = grp_pool.tile([128, nw, C], F32, name="dd", tag="dd")
                nc.vector.tensor_tensor(
                    out=dd[:], in0=h_g[:],
                    in1=mx2[:].unsqueeze(2).to_broadcast([128, nw, C]),
                    op=ALU.subtract)
                exd = grp_pool.tile([128, nw, C], F32, name="exd", tag="exd")
                nc.scalar.activation(exd[:], dd[:], AF.Exp)
                s2 = sm_pool.tile([128, nw], F32, name="s2", tag="ssum")
                nc.vector.tensor_reduce(s2[:], exd[:], axis=AX.X, op=ALU.add)
                ln2 = sm_pool.tile([128, nw], F32, name="ln2", tag="rs")
                nc.scalar.activation(ln2[:], s2[:], AF.Ln)
                ot = grp_pool.tile([128, nw, C], F32, name="ot", tag="ot")
                nc.vector.tensor_tensor(
                    out=ot[:], in0=dd[:],
                    in1=ln2[:].unsqueeze(2).to_broadcast([128, nw, C]),
                    op=ALU.subtract)
                nc.sync.dma_start(out_ap[:, ws[0]:ws[0] + nw, :], ot[:])

            run_layer(2, t2_full, C, xh2_res, omlp2_res, attl2_sb, avec2_sb,
                      sink2)

        for _rep in range(repeat):
            emit_once()
        es.close()

    nc.compile()
    return nc


# --------------------------------------------------------------------------
# Runner (cached compiled program + jitted PJRT executable)
# --------------------------------------------------------------------------

_CACHE = {}


class _Runner:
    def __init__(self, plan, cfg):
        self.cfg = cfg
        self.plan = plan
        self.nc = build_program(plan, cfg)
        self._fn = None

    def _build_fn(self):
        import jax
        from jax.sharding import Mesh, PartitionSpec
        from jax.experimental.shard_map import shard_map
        from concourse import bass2jax

        nc = self.nc
        NC = self.cfg["NC"]
        bass2jax.install_neuronx_cc_hook()
        partition_name = (nc.partition_id_tensor.name
                          if nc.partition_id_tensor else None)
        in_names, out_names, out_avals, zero_outs = [], [], [], []
        for alloc in nc.m.functions[0].allocations:
            if not isinstance(alloc, mybir.MemoryLocationSet):
                continue
            name = alloc.memorylocations[0].name
            if alloc.kind == "ExternalInput":
                if name != partition_name:
                    in_names.append(name)
            elif alloc.kind == "ExternalOutput":
                shape = tuple(alloc.tensor_shape)
                dtype = mybir.dt.np(alloc.dtype)
                out_avals.append(jax.core.ShapedArray(shape, dtype))
                out_names.append(name)
                zero_outs.append(np.zeros(shape, dtype))
        n_params = len(in_names)
        bind_in_names = list(in_names) + list(out_names)
        if partition_name is not None:
            bind_in_names.append(partition_name)

        def _body(*args):
            operands = list(args)
            if partition_name is not None:
                operands.append(bass2jax.partition_id_tensor())
            outs = bass2jax._bass_exec_p.bind(
                *operands,
                out_avals=tuple(out_avals),
                in_names=tuple(bind_in_names),
                out_names=tuple(out_names),
                lowering_input_output_aliases=(),
                sim_require_finite=True,
                sim_require_nnan=True,
                nc=nc,
            )
            return tuple(outs)

        devices = jax.devices()[:NC]
        mesh = Mesh(np.asarray(devices), ("core",))
        n_outs = len(out_names)
        in_specs = (PartitionSpec("core"),) * (n_params + n_outs)
        out_specs = (PartitionSpec("core"),) * n_outs
        fn = jax.jit(
            shard_map(_body, mesh=mesh, in_specs=in_specs,
                      out_specs=out_specs, check_rep=False),
            keep_unused=True)
        self._fn = fn
        self._in_names = in_names
        self._out_names = out_names
        self._out_avals = out_avals
        self._zero_outs = zero_outs

    def prepare_args(self, in_maps):
        import jax
        NC = self.cfg["NC"]
        per_core = [[np.asarray(m[name]) for name in self._in_names]
                    for m in in_maps]
        concat_in = [np.concatenate([per_core[c][i] for c in range(NC)], axis=0)
                     for i in range(len(self._in_names))]
        concat_zeros = [np.zeros((NC * z.shape[0], *z.shape[1:]), z.dtype)
                        for z in self._zero_outs]
        return [jax.device_put(a) for a in concat_in + concat_zeros]

    def time_ns(self, in_maps, r_hi=6, reps=40):
        """Per-execution device time, measured by differencing wall times of
        this NEFF vs a variant whose body repeats the whole kernel r_hi times
        (fixed RPC + input-staging costs cancel in the difference)."""
        import time
        import jax

        if self._fn is None:
            self._build_fn()
        if not hasattr(self, "_fn_hi") or self._fn_hi is None:
            rh = _Runner.__new__(_Runner)
            rh.cfg = self.cfg
            rh.plan = self.plan
            rh.nc = build_program(self.plan, self.cfg, repeat=r_hi)
            rh._fn = None
            rh._build_fn()
            self._fn_hi = rh._fn
            self._rh = rh
            self._r_hi = r_hi
        a1 = self.prepare_args(in_maps)
        ah = self._rh.prepare_args(in_maps)
        jax.block_until_ready(self._fn(*a1))
        jax.block_until_ready(self._fn_hi(*ah))
        t1s, ths = [], []
        for _ in range(reps):
            t0 = time.perf_counter()
            jax.block_until_ready(self._fn(*a1))
            t1s.append(time.perf_counter() - t0)
            t0 = time.perf_counter()
            jax.block_until_ready(self._fn_hi(*ah))
            ths.append(time.perf_counter() - t0)
        t1s.sort()
        ths.sort()
        i = max(1, reps // 10)
        return (ths[i] - t1s[i]) / (self._r_hi - 1) * 1e9

    def run(self, in_maps):
        import jax
        if self._fn is None:
            self._build_fn()
        args = self.prepare_args(in_maps)
        outs = self._fn(*args)
        jax.block_until_ready(outs)
        NC = self.cfg["NC"]
        res = []
        for c in range(NC):
            m = {}
            for i, name in enumerate(self._out_names):
                m[name] = np.asarray(outs[i]).reshape(
                    NC, *self._out_avals[i].shape)[c]
            res.append(m)
        return res


def get_runner(inputs, cfg=None):
    cfg = dict(DEFAULT_CFG if cfg is None else cfg)
    plan = make_plan(np.asarray(inputs["edge_row"]).astype(np.int64),
                     np.asarray(inputs["edge_col"]).astype(np.int64), cfg)
    key = plan_key(plan, cfg)
    if key not in _CACHE:
        _CACHE[key] = _Runner(plan, cfg)
    return _CACHE[key], plan


def kernel(**inputs) -> np.ndarray:
    cfg = dict(DEFAULT_CFG)
    runner, plan = get_runner(inputs, cfg)
    in_maps = pack_inputs(inputs, runner.plan, cfg)
    res = runner.run(in_maps)
    NSH = runner.plan["d"]["NSH"]
    out = np.concatenate([res[k]["out"][:NSH] for k in range(cfg["NC"])],
                         axis=0)
    return out[:cfg["N"]].astype(np.float32)


_CALIB = {}


def _calib_runner(n_cores):
    """Tiny NEFF used to measure the per-execution dispatch overhead."""
    if n_cores in _CALIB:
        return _CALIB[n_cores]
    import jax
    import concourse.tile as tile_mod

    nc = bacc.Bacc(None, target_bir_lowering=False, num_devices=n_cores)
    inp = nc.declare_dram_parameter("inp", [128, 64], F32, isOutput=False)
    out = nc.declare_dram_parameter("out", [128, 64], F32, isOutput=True)
    with tile_mod.TileContext(nc) as tc:
        with tc.tile_pool(name="sbuf", bufs=2) as sbuf:
            t = sbuf.tile([128, 64], F32, name="t")
            nc.sync.dma_start(t[:], inp[:])
            nc.vector.tensor_scalar(t[:], t[:], 2.0, None, ALU.mult)
            nc.sync.dma_start(out[:], t[:])
    nc.compile()

    class _Tmp:
        pass

    r = _Runner.__new__(_Runner)
    r.cfg = dict(NC=n_cores)
    r.nc = nc
    r._fn = None
    r._build_fn()
    x = np.zeros((128, 64), np.float32)
    args = r.prepare_args([{"inp": x} for _ in range(n_cores)])
    _CALIB[n_cores] = (r._fn, args)
    return _CALIB[n_cores]



# revision 19
# speedup vs baseline: 1.5244x; 1.3207x over previous
"""ACM-GCN (2-layer) distributed Bass kernel for 8 TRN2 NeuronCores.

Strategy:
  - Shard nodes (rows of x / segment ids) across 8 cores: core k owns global
    rows [k*6250, (k+1)*6250), padded to 6272 = 49*128 per core.
  - Layer matmuls (x @ W_*) are local per core (lhsT = pre-transposed x tiles
    shipped from host in bf16).
  - SpMM (adj_low @ xl etc): each core owns the edges whose DESTINATION row
    lives on it. The [xl|xh] activations are cast to fp8e4 and AllGathered
    into a per-core HBM table; per 128-row destination window the kernel
    dma_gathers the source rows (fp8, 256B rows) and segment-sums them with
    one-hot val matmuls accumulated in PSUM. The one-hot lhsT matrices are
    precomputed on the host in fp8 (carrying 64*val to stay in e4m3 normal
    range; the 1/64 is folded into the post-PSUM relu scale) and streamed
    from HBM, so no engine spends time building them.
  - Attention + combine + log_softmax are node-parallel (local), batched at
    layer granularity to minimize DVE instruction count.

All metadata (edge->window assignment, gather indices, one-hot matrices) is
packed host-side in numpy; the Bass graph is static with data-dependent chunk
capacities shared across all 8 cores (max over cores).
"""

import math

import numpy as np
import ml_dtypes

import concourse.bass as bass
import concourse.mybir as mybir
import concourse.tile as tile
from concourse import bacc
from concourse.masks import make_identity

F32 = mybir.dt.float32
BF16 = mybir.dt.bfloat16
F8 = mybir.dt.float8e4
I16 = mybir.dt.int16
BF = ml_dtypes.bfloat16
NF8 = ml_dtypes.float8_e4m3
AF = mybir.ActivationFunctionType
ALU = mybir.AluOpType
AX = mybir.AxisListType

VSCALE = 64.0

DEFAULT_CFG = dict(N=50000, F=512, H=128, C=64, NC=8, GROUP=4, NQ=4)


# --------------------------------------------------------------------------
# Host-side planning / packing
# --------------------------------------------------------------------------

def derive(cfg):
    N, F, H, C, NC = cfg["N"], cfg["F"], cfg["H"], cfg["C"], cfg["NC"]
    assert N % NC == 0
    NSH = N // NC
    PW = (NSH + 127) // 128
    NPAD = PW * 128
    TBL = NC * NPAD
    HALF = TBL // 2
    assert HALF <= 32768, "int16 gather index limit"
    assert F % 128 == 0
    FK = F // 128
    return dict(NSH=NSH, PW=PW, NPAD=NPAD, TBL=TBL, HALF=HALF, FK=FK)


def make_plan(edge_row, edge_col, cfg):
    """Static shape plan shared by all cores: chunk counts per (window, half),
    grouping, chunk bases."""
    d = derive(cfg)
    NC, GROUP = cfg["NC"], cfg["GROUP"]
    NSH, PW, NPAD, HALF = d["NSH"], d["PW"], d["NPAD"], d["HALF"]

    core = edge_row // NSH
    dr = edge_row - core * NSH
    w = dr // 128
    tcol = (edge_col // NSH) * NPAD + (edge_col % NSH)
    half = (tcol >= HALF).astype(np.int64)
    key = (core.astype(np.int64) * PW + w) * 2 + half
    counts = np.bincount(key, minlength=NC * PW * 2).reshape(NC, PW, 2)
    gchunks = (counts.max(axis=0) + 127) // 128  # [PW, 2]

    groups = [list(range(g, min(g + GROUP, PW))) for g in range(0, PW, GROUP)]
    chunk_base = {}
    gc = 0
    calls = []  # per group: [(half, total_chunks, base_chunk)]
    for ws in groups:
        gcalls = []
        for hf in (0, 1):
            base = gc
            for wi in ws:
                chunk_base[(wi, hf)] = gc
                gc += int(gchunks[wi, hf])
            gcalls.append((hf, gc - base, base))
        calls.append(gcalls)
    return dict(d=d, gchunks=gchunks, groups=groups, chunk_base=chunk_base,
                calls=calls, GC=gc)


def plan_key(plan, cfg):
    return (tuple(sorted(cfg.items())),
            tuple(map(int, plan["gchunks"].flatten())))


def pack_inputs(inputs, plan, cfg):
    """Build per-core in_maps (numpy) for the bass program."""
    d = plan["d"]
    N, F, H, C, NC = cfg["N"], cfg["F"], cfg["H"], cfg["C"], cfg["NC"]
    NSH, PW, NPAD, HALF, FK = d["NSH"], d["PW"], d["NPAD"], d["HALF"], d["FK"]
    GC = plan["GC"]
    chunk_base = plan["chunk_base"]

    x = np.asarray(inputs["x"], np.float32)
    er = np.asarray(inputs["edge_row"]).astype(np.int64)
    ec = np.asarray(inputs["edge_col"]).astype(np.int64)
    ev = np.asarray(inputs["edge_val"], np.float32)

    w1 = np.concatenate([np.asarray(inputs["weight_low"], np.float32),
                         np.asarray(inputs["weight_high"], np.float32),
                         np.asarray(inputs["weight_mlp"], np.float32)], axis=1)  # [F, 3H]
    w2 = np.concatenate([np.asarray(inputs["weight_low2"], np.float32),
                         np.asarray(inputs["weight_high2"], np.float32),
                         np.asarray(inputs["weight_mlp2"], np.float32)], axis=1)  # [H, 3C]
    # w1 sbuf layout [128, FK, 3H]: [p, kk, n] = w1[kk*128+p, n]
    w1_sb = np.ascontiguousarray(
        w1.reshape(FK, 128, 3 * H).transpose(1, 0, 2)).astype(BF)
    w2_sb = w2.astype(BF)  # [H=128, 3C]

    attl1 = np.stack([np.asarray(inputs["att_vec_low"], np.float32)[:, 0],
                      np.asarray(inputs["att_vec_high"], np.float32)[:, 0],
                      np.asarray(inputs["att_vec_mlp"], np.float32)[:, 0]])  # [3, H]
    attl2 = np.stack([np.asarray(inputs["att_vec_low2"], np.float32)[:, 0],
                      np.asarray(inputs["att_vec_high2"], np.float32)[:, 0],
                      np.asarray(inputs["att_vec_mlp2"], np.float32)[:, 0]])  # [3, C]
    attl1_sb = np.broadcast_to(attl1[None], (128, 3, H)).astype(BF).copy()
    attl2_sb = np.broadcast_to(attl2[None], (128, 3, C)).astype(BF).copy()
    avec1 = (np.asarray(inputs["att_vec"], np.float32) / 3.0).reshape(9)
    avec2 = (np.asarray(inputs["att_vec2"], np.float32) / 3.0).reshape(9)
    avec1_sb = np.broadcast_to(avec1[None], (128, 9)).astype(np.float32).copy()
    avec2_sb = np.broadcast_to(avec2[None], (128, 9)).astype(np.float32).copy()

    # edge metadata
    core = er // NSH
    dr = er - core * NSH
    w = dr // 128
    rl = (dr % 128).astype(np.int64)
    tcol = (ec // NSH) * NPAD + (ec % NSH)
    half = (tcol >= HALF).astype(np.int64)
    idxv = (tcol - half * HALF).astype(np.int64)

    in_maps = []
    for k in range(NC):
        sel = core == k
        kw, khalf, kidx, krl, kval = w[sel], half[sel], idxv[sel], rl[sel], ev[sel]
        # order edges by (w, half)
        order = np.lexsort((khalf, kw))
        kw, khalf, kidx, krl, kval = (a[order] for a in (kw, khalf, kidx, krl, kval))
        seg_key = kw * 2 + khalf
        seg_counts = np.bincount(seg_key, minlength=PW * 2).reshape(PW, 2)
        # global slot for each edge: chunk_base[(w,half)]*128 + position-in-segment
        starts = np.zeros(PW * 2, np.int64)
        np.cumsum(seg_counts.flatten()[:-1], out=starts[1:])
        pos_in_seg = np.arange(len(kw)) - starts[seg_key]
        cb = np.array([[chunk_base[(wi, hf)] for hf in (0, 1)] for wi in range(PW)],
                      np.int64)
        slot = cb[kw, khalf] * 128 + (pos_in_seg // 128) * 128 + pos_in_seg % 128

        idx_flat = np.zeros(GC * 128, np.int16)
        idx_flat[slot] = kidx.astype(np.int16)
        p = slot % 128
        c = slot // 128
        # one-hot matmul lhsT matrices, host-precomputed in fp8:
        # oh[p, c, j] = VSCALE * val for the edge at slot (c*128+p) with local
        # dest row j; zero elsewhere.
        oh = np.zeros((128, GC, 128), NF8)
        oh[p, c, krl] = (kval * VSCALE).astype(NF8)
        # idxs layout for dma_gather: [j%16, j//16] replicated over 8 groups of
        # 16 partitions
        idx16 = np.tile(idx_flat.reshape(-1, 16).T, (8, 1))  # [128, GC*8]

        # pre-transposed x tiles: [PW, 128, FK, 128]:
        # xt[m, p, kk, j] = x[k*NSH + m*128 + j, kk*128 + p]
        xk = np.zeros((NPAD, F), np.float32)
        xk[:NSH] = x[k * NSH:(k + 1) * NSH]
        xt = np.ascontiguousarray(
            xk.reshape(PW, 128, FK, 128).transpose(0, 3, 2, 1)).astype(BF)

        in_maps.append({
            "xt": xt,
            "w1s": w1_sb, "w2s": w2_sb,
            "attl1": attl1_sb, "attl2": attl2_sb,
            "avec1": avec1_sb, "avec2": avec2_sb,
            "idx16": idx16,
            "oh": oh,
        })
    return in_maps


# --------------------------------------------------------------------------
# Bass program
# --------------------------------------------------------------------------

def build_program(plan, cfg, repeat=1):
    d = plan["d"]
    N, F, H, C, NC = cfg["N"], cfg["F"], cfg["H"], cfg["C"], cfg["NC"]
    NSH, PW, NPAD, TBL, HALF, FK = (d["NSH"], d["PW"], d["NPAD"], d["TBL"],
                                    d["HALF"], d["FK"])
    GC = plan["GC"]
    gchunks = plan["gchunks"]
    groups = plan["groups"]
    chunk_base = plan["chunk_base"]
    calls = plan["calls"]
    H2, C2, C3 = 2 * H, 2 * C, 3 * C
    H3 = 3 * H
    TW = 256  # fp8 table row width (elems) for both layers (L2 is padded)
    IVS = 1.0 / VSCALE

    nc = bacc.Bacc(None, target_bir_lowering=False, num_devices=NC,
                   num_swdge_queues=int(cfg.get("NQ", 1)))

    xt_d = nc.declare_dram_parameter("xt", [PW, 128, FK, 128], BF16, isOutput=False)
    w1_d = nc.declare_dram_parameter("w1s", [128, FK, H3], BF16, isOutput=False)
    w2_d = nc.declare_dram_parameter("w2s", [H, C3], BF16, isOutput=False)
    attl1_d = nc.declare_dram_parameter("attl1", [128, 3, H], BF16, isOutput=False)
    attl2_d = nc.declare_dram_parameter("attl2", [128, 3, C], BF16, isOutput=False)
    avec1_d = nc.declare_dram_parameter("avec1", [128, 9], F32, isOutput=False)
    avec2_d = nc.declare_dram_parameter("avec2", [128, 9], F32, isOutput=False)
    idx16_d = nc.declare_dram_parameter("idx16", [128, GC * 8], I16, isOutput=False)
    oh_d = nc.declare_dram_parameter("oh", [128, GC, 128], F8, isOutput=False)
    out_d = nc.declare_dram_parameter("out", [NPAD, C], F32, isOutput=True)

    with tile.TileContext(nc) as tc:
        # ---- DRAM internals
        from contextlib import ExitStack
        es = ExitStack()
        dram_pool = es.enter_context(
            tc.tile_pool(name="dram_pool", bufs=1, space="DRAM"))

        # ---- resident constants / metadata
        consts = es.enter_context(tc.tile_pool(name="consts", bufs=1))
        w1_sb = consts.tile([128, FK, H3], BF16, name="w1_sb")
        nc.sync.dma_start(w1_sb[:], w1_d[:])
        w2_sb = consts.tile([H, C3], BF16, name="w2_sb")
        nc.sync.dma_start(w2_sb[:], w2_d[:])
        attl1_sb = consts.tile([128, 3, H], BF16, name="attl1_sb")
        nc.sync.dma_start(attl1_sb[:], attl1_d[:])
        attl2_sb = consts.tile([128, 3, C], BF16, name="attl2_sb")
        nc.sync.dma_start(attl2_sb[:], attl2_d[:])
        avec1_sb = consts.tile([128, 9], F32, name="avec1_sb")
        nc.sync.dma_start(avec1_sb[:], avec1_d[:])
        avec2_sb = consts.tile([128, 9], F32, name="avec2_sb")
        nc.sync.dma_start(avec2_sb[:], avec2_d[:])
        idx_sb = consts.tile([128, GC * 8], I16, name="idx_sb")
        nc.sync.dma_start(idx_sb[:], idx16_d[:])
        ident = consts.tile([128, 128], BF16, name="ident")
        make_identity(nc, ident[:])

        # ---- resident activations
        res = es.enter_context(tc.tile_pool(name="res", bufs=1))
        xh1_res = res.tile([128, PW, H], BF16, name="xh1_res")
        omlp1_res = res.tile([128, PW, H], BF16, name="omlp1_res")
        xh2_res = res.tile([128, PW, C], BF16, name="xh2_res")
        omlp2_res = res.tile([128, PW, C], BF16, name="omlp2_res")
        olow1_a = res.tile([128, PW, H], BF16, name="olow1_a")
        ohigh1_a = res.tile([128, PW, H], BF16, name="ohigh1_a")
        olow2_a = res.tile([128, PW, C], BF16, name="olow2_a")
        ohigh2_a = res.tile([128, PW, C], BF16, name="ohigh2_a")
        feats1_a = res.tile([128, PW, 3], F32, name="feats1_a")
        feats2_a = res.tile([128, PW, 3], F32, name="feats2_a")
        hc1_a = res.tile([128, PW, H], BF16, name="hc1_a")
        hc2_a = res.tile([128, PW, C], BF16, name="hc2_a")

        # ---- pools
        xt_pool = es.enter_context(tc.tile_pool(name="xt_pool", bufs=3))
        ps1_pool = es.enter_context(tc.tile_pool(name="ps1", bufs=2, space="PSUM"))
        psw_pool = es.enter_context(tc.tile_pool(name="psw", bufs=2, space="PSUM"))
        psT_pool = es.enter_context(tc.tile_pool(name="psT", bufs=2, space="PSUM"))
        ps2_pool = es.enter_context(tc.tile_pool(name="ps2", bufs=2, space="PSUM"))
        g_pool = es.enter_context(tc.tile_pool(name="g_pool", bufs=2))
        oh_pool = es.enter_context(tc.tile_pool(name="oh_pool", bufs=2))
        wtmp_pool = es.enter_context(tc.tile_pool(name="wtmp", bufs=3))
        sm_pool = es.enter_context(tc.tile_pool(name="sm", bufs=2))

        rep_ctr = [0]

        def emit_once():
            rep = rep_ctr[0]
            rep_ctr[0] += 1
            qrr = [0]  # round-robin SWDGE queue counter for gathers
            t1_local = dram_pool.tile([NPAD, TW], F8, name="t1_local",
                                      tag=f"t1l{rep}")
            t1_full = dram_pool.tile([TBL, TW], F8, name="t1_full",
                                     tag=f"t1f{rep}", addr_space="Shared")
            t2_local = dram_pool.tile([NPAD, TW], F8, name="t2_local",
                                      tag=f"t2l{rep}")
            t2_full = dram_pool.tile([TBL, TW], F8, name="t2_full",
                                     tag=f"t2f{rep}", addr_space="Shared")

            # ================= Phase A: layer-1 local matmuls =================
            for m in range(PW):
                xt_t = xt_pool.tile([128, FK, 128], BF16, name="xt_t")
                nc.sync.dma_start(xt_t[:], xt_d[m])
                ps = ps1_pool.tile([128, H3], F32, name="ps1_t")
                for kk in range(FK):
                    nc.tensor.matmul(out=ps[:], lhsT=xt_t[:, kk, :],
                                     rhs=w1_sb[:, kk, :],
                                     start=(kk == 0), stop=(kk == FK - 1))
                # [xl|xh] -> fp8 table + local copies
                t1w = wtmp_pool.tile([128, TW], F8, name="t1w", tag="t1w")
                nc.scalar.copy(t1w[:], ps[:, 0:H2])
                nc.sync.dma_start(t1_local[m * 128:(m + 1) * 128, :], t1w[:])
                nc.scalar.activation(xh1_res[:, m, :], ps[:, H:H2], AF.Copy,
                                     scale=VSCALE)
                nc.scalar.activation(omlp1_res[:, m, :], ps[:, H2:H3], AF.Relu)

            # ================= Phase B: AllGather 1 =================
            nc.gpsimd.collective_compute(
                "AllGather", ALU.bypass,
                replica_groups=[list(range(NC))],
                ins=[t1_local[:].opt()],
                outs=[t1_full[:].opt()],
            )

            # ---- generic window-loop machinery (shared by both layers) ----
            def run_layer(layer, table, ew, xh_res, omlp_res, olow_a, ohigh_a,
                          feats_a, hc_a, attl_sb, avec_sb, sink):
                """layer: 1 or 2; table: DRAM tile [TBL, TW]; ew: H or C.
                xh_res holds VSCALE*xh; olow/ohigh land at true scale."""
                ew2 = 2 * ew
                nq = int(cfg.get("NQ", 1))
                for gi, ws in enumerate(groups):
                    nw = len(ws)
                    # gathers for this group (one call per half, rr queues)
                    g_tiles = {}
                    for (hf, nch, base) in calls[gi]:
                        if nch == 0:
                            continue
                        gt = g_pool.tile([128, nch, TW], F8,
                                         name=f"g{layer}_{hf}", tag=f"gt{hf}")
                        src = table[0:HALF, :] if hf == 0 else table[HALF:TBL, :]
                        nc.gpsimd.dma_gather(
                            gt[:, :, :], src,
                            idx_sb[:, base * 8:(base + nch) * 8],
                            nch * 128, nch * 128, TW,
                            single_packet=bool(cfg.get("SP1", False)),
                            queue_num=qrr[0] % nq)
                        qrr[0] += 1
                        g_tiles[hf] = (gt, base)
                    # stream this group's one-hot lhsT matrices from HBM
                    gbase = calls[gi][0][2]
                    gtot = sum(nch for (_hf, nch, _b) in calls[gi])
                    oh_g = oh_pool.tile([128, gtot, 128], F8, name="oh_g",
                                        tag="oh_g")
                    nc.sync.dma_start(oh_g[:], oh_d[:, gbase:gbase + gtot, :])
                    for wi, w in enumerate(ws):
                        ps = psw_pool.tile([128, ew2], F32, name="psw_t")
                        spans = []
                        for hf in (0, 1):
                            nch_w = int(gchunks[w, hf])
                            if nch_w == 0 or hf not in g_tiles:
                                continue
                            gt, base = g_tiles[hf]
                            cb = chunk_base[(w, hf)]
                            spans.append((gt, cb - base, cb, nch_w))
                        total = sum(s[3] for s in spans)
                        ci = 0
                        for (gt, loff, gcb, nch_w) in spans:
                            for c in range(nch_w):
                                gcc = gcb + c
                                nc.tensor.matmul(out=ps[:],
                                                 lhsT=oh_g[:, gcc - gbase, :],
                                                 rhs=gt[:, loff + c, 0:ew2],
                                                 start=(ci == 0),
                                                 stop=(ci == total - 1))
                                ci += 1
                        if total == 0:
                            nc.vector.memset(ps[:], 0.0)
                        # o_low = relu(S_low) = relu(ps_low) / VSCALE
                        nc.scalar.activation(olow_a[:, w, :], ps[:, 0:ew],
                                             AF.Relu, scale=IVS)
                        # o_high = relu(xh - S_high) = relu(xh64 - ps_high)/VS
                        tmp = wtmp_pool.tile([128, ew], F32, name="ohtmp",
                                             tag="ohtmp")
                        nc.vector.tensor_tensor(out=tmp[:],
                                                in0=xh_res[:, w, :],
                                                in1=ps[:, ew:ew2],
                                                op=ALU.subtract)
                        nc.scalar.activation(ohigh_a[:, w, :], tmp[:],
                                             AF.Relu, scale=IVS)
                    # attention feats (batched per group)
                    pr = wtmp_pool.tile([128, nw, 3, ew], BF16, name="attn_pr",
                                        tag="attn_pr")
                    for j, src_t in enumerate(
                            (olow_a[:, ws[0]:ws[0] + nw, :],
                             ohigh_a[:, ws[0]:ws[0] + nw, :],
                             omlp_res[:, ws[0]:ws[0] + nw, :])):
                        nc.vector.tensor_tensor(
                            out=pr[:, :, j, :], in0=src_t,
                            in1=attl_sb[:, j, :].unsqueeze(1)
                                .to_broadcast([128, nw, ew]),
                            op=ALU.mult)
                    nc.vector.tensor_reduce(feats_a[:, ws[0]:ws[0] + nw, :],
                                            pr[:], axis=AX.X, op=ALU.add)

                # ---- attention (batched per layer) ----
                sig = sm_pool.tile([128, PW, 3], F32, name="sig", tag="sig")
                nc.scalar.activation(sig[:], feats_a[:], AF.Sigmoid)
                zat = sm_pool.tile([128, PW, 3], F32, name="zat", tag="zat")
                za = sm_pool.tile([128, PW], F32, name="za", tag="za")
                zb = sm_pool.tile([128, PW], F32, name="zb", tag="zb")
                for j in range(3):
                    nc.vector.tensor_scalar(za[:], sig[:, :, 0],
                                            avec_sb[:, 0 + j:1 + j], None,
                                            ALU.mult)
                    nc.vector.tensor_scalar(zb[:], sig[:, :, 1],
                                            avec_sb[:, 3 + j:4 + j], None,
                                            ALU.mult)
                    nc.vector.tensor_tensor(out=za[:], in0=za[:], in1=zb[:],
                                            op=ALU.add)
                    nc.vector.tensor_scalar(zb[:], sig[:, :, 2],
                                            avec_sb[:, 6 + j:7 + j], None,
                                            ALU.mult)
                    nc.vector.tensor_tensor(out=zat[:, :, j], in0=za[:],
                                            in1=zb[:], op=ALU.add)
                mx = sm_pool.tile([128, PW], F32, name="mx", tag="mx")
                nc.vector.tensor_reduce(mx[:], zat[:], axis=AX.X, op=ALU.max)
                zs = sm_pool.tile([128, PW, 3], F32, name="zs", tag="zs")
                nc.vector.tensor_tensor(
                    out=zs[:], in0=zat[:],
                    in1=mx[:].unsqueeze(2).to_broadcast([128, PW, 3]),
                    op=ALU.subtract)
                ez = sm_pool.tile([128, PW, 3], F32, name="ez", tag="ez")
                nc.scalar.activation(ez[:], zs[:], AF.Exp)
                ssum = sm_pool.tile([128, PW], F32, name="ssum", tag="ssum")
                nc.vector.tensor_reduce(ssum[:], ez[:], axis=AX.X, op=ALU.add)
                rs = sm_pool.tile([128, PW], F32, name="rs", tag="rs")
                nc.vector.reciprocal(rs[:], ssum[:])
                nc.vector.tensor_scalar(rs[:], rs[:], 3.0, None, ALU.mult)
                att = sm_pool.tile([128, PW, 3], F32, name="att", tag="att")
                nc.vector.tensor_tensor(
                    out=att[:], in0=ez[:],
                    in1=rs[:].unsqueeze(2).to_broadcast([128, PW, 3]),
                    op=ALU.mult)
                # ---- combine (batched per layer, in place: olow/ohigh
                # are dead once feats are computed) ----
                nc.vector.tensor_tensor(
                    out=olow_a[:], in0=olow_a[:],
                    in1=att[:, :, 0].unsqueeze(2).to_broadcast([128, PW, ew]),
                    op=ALU.mult)
                nc.vector.tensor_tensor(
                    out=ohigh_a[:], in0=ohigh_a[:],
                    in1=att[:, :, 1].unsqueeze(2).to_broadcast([128, PW, ew]),
                    op=ALU.mult)
                nc.vector.tensor_tensor(out=olow_a[:], in0=olow_a[:],
                                        in1=ohigh_a[:], op=ALU.add)
                nc.vector.tensor_tensor(
                    out=ohigh_a[:], in0=omlp_res[:],
                    in1=att[:, :, 2].unsqueeze(2).to_broadcast([128, PW, ew]),
                    op=ALU.mult)
                nc.vector.tensor_tensor(out=hc_a[:], in0=olow_a[:],
                                        in1=ohigh_a[:], op=ALU.add)
                sink(hc_a)

            # ================= Phase C: layer-1 windows =================
            def sink1(h_a):
                # transpose h per window, layer-2 local matmul
                for w in range(PW):
                    psT = psT_pool.tile([128, H], BF16, name="psT_t")
                    nc.tensor.transpose(psT[:], h_a[:, w, :], ident[:])
                    hT = wtmp_pool.tile([128, H], BF16, name="hT", tag="hT")
                    nc.scalar.copy(hT[:], psT[:])
                    ps2 = ps2_pool.tile([128, C3], F32, name="ps2_t")
                    nc.tensor.matmul(out=ps2[:], lhsT=hT[:], rhs=w2_sb[:],
                                     start=True, stop=True)
                    t2w = wtmp_pool.tile([128, TW], F8, name="t2w", tag="t2w")
                    nc.scalar.copy(t2w[:, 0:C2], ps2[:, 0:C2])
                    nc.scalar.copy(t2w[:, C2:TW], ps2[:, 0:C2])
                    nc.sync.dma_start(t2_local[w * 128:(w + 1) * 128, :],
                                      t2w[:])
                    nc.scalar.activation(xh2_res[:, w, :], ps2[:, C:C2],
                                         AF.Copy, scale=VSCALE)
                    nc.scalar.activation(omlp2_res[:, w, :], ps2[:, C2:C3],
                                         AF.Relu)

            run_layer(1, t1_full, H, xh1_res, omlp1_res, olow1_a, ohigh1_a,
                      feats1_a, hc1_a, attl1_sb, avec1_sb, sink1)

            # ================= Phase D: AllGather 2 =================
            nc.gpsimd.collective_compute(
                "AllGather", ALU.bypass,
                replica_groups=[list(range(NC))],
                ins=[t2_local[:].opt()],
                outs=[t2_full[:].opt()],
            )

            # ================= Phase E: layer-2 windows + log_softmax ========
            out_ap = out_d[:].rearrange("(w p) c -> p w c", p=128)

            def sink2(h_a):
                mx2 = sm_pool.tile([128, PW], F32, name="mx2", tag="mx")
                nc.vector.tensor_reduce(mx2[:], h_a[:], axis=AX.X, op=ALU.max)
                for gi, ws in enumerate(groups):
                    w0, nw = ws[0], len(ws)
                    dd = sm_pool.tile([128, len(groups[0]), C], F32,
                                      name="dd", tag="dd")
                    ddv = dd[:, 0:nw, :]
                    nc.vector.tensor_tensor(
                        out=ddv, in0=h_a[:, w0:w0 + nw, :],
                        in1=mx2[:, w0:w0 + nw].unsqueeze(2)
                            .to_broadcast([128, nw, C]),
                        op=ALU.subtract)
                    exd = sm_pool.tile([128, len(groups[0]), C], F32,
                                       name="exd", tag="exd")
                    exv = exd[:, 0:nw, :]
                    nc.scalar.activation(exv, ddv, AF.Exp)
                    s2 = sm_pool.tile([128, len(groups[0])], F32, name="s2",
                                      tag="s2")
                    nc.vector.tensor_reduce(s2[:, 0:nw], exv, axis=AX.X,
                                            op=ALU.add)
                    ln2 = sm_pool.tile([128, len(groups[0])], F32, name="ln2",
                                       tag="ln2")
                    nc.scalar.activation(ln2[:, 0:nw], s2[:, 0:nw], AF.Ln)
                    nc.vector.tensor_tensor(
                        out=ddv, in0=ddv,
                        in1=ln2[:, 0:nw].unsqueeze(2).to_broadcast([128, nw, C]),
                        op=ALU.subtract)
                    nc.sync.dma_start(out_ap[:, w0:w0 + nw, :], ddv)

            run_layer(2, t2_full, C, xh2_res, omlp2_res, olow2_a, ohigh2_a,
                      feats2_a, hc2_a, attl2_sb, avec2_sb, sink2)

        for _rep in range(repeat):
            emit_once()
        es.close()

    nc.compile()
    return nc


# --------------------------------------------------------------------------
# Runner (cached compiled program + jitted PJRT executable)
# --------------------------------------------------------------------------

_CACHE = {}


class _Runner:
    def __init__(self, plan, cfg):
        self.cfg = cfg
        self.plan = plan
        self.nc = build_program(plan, cfg)
        self._fn = None

    def _build_fn(self):
        import jax
        from jax.sharding import Mesh, PartitionSpec
        from jax.experimental.shard_map import shard_map
        from concourse import bass2jax

        nc = self.nc
        NC = self.cfg["NC"]
        bass2jax.install_neuronx_cc_hook()
        partition_name = (nc.partition_id_tensor.name
                          if nc.partition_id_tensor else None)
        in_names, out_names, out_avals, zero_outs = [], [], [], []
        for alloc in nc.m.functions[0].allocations:
            if not isinstance(alloc, mybir.MemoryLocationSet):
                continue
            name = alloc.memorylocations[0].name
            if alloc.kind == "ExternalInput":
                if name != partition_name:
                    in_names.append(name)
            elif alloc.kind == "ExternalOutput":
                shape = tuple(alloc.tensor_shape)
                dtype = mybir.dt.np(alloc.dtype)
                out_avals.append(jax.core.ShapedArray(shape, dtype))
                out_names.append(name)
                zero_outs.append(np.zeros(shape, dtype))
        n_params = len(in_names)
        bind_in_names = list(in_names) + list(out_names)
        if partition_name is not None:
            bind_in_names.append(partition_name)

        def _body(*args):
            operands = list(args)
            if partition_name is not None:
                operands.append(bass2jax.partition_id_tensor())
            outs = bass2jax._bass_exec_p.bind(
                *operands,
                out_avals=tuple(out_avals),
                in_names=tuple(bind_in_names),
                out_names=tuple(out_names),
                lowering_input_output_aliases=(),
                sim_require_finite=True,
                sim_require_nnan=True,
                nc=nc,
            )
            return tuple(outs)

        devices = jax.devices()[:NC]
        mesh = Mesh(np.asarray(devices), ("core",))
        n_outs = len(out_names)
        in_specs = (PartitionSpec("core"),) * (n_params + n_outs)
        out_specs = (PartitionSpec("core"),) * n_outs
        fn = jax.jit(
            shard_map(_body, mesh=mesh, in_specs=in_specs,
                      out_specs=out_specs, check_rep=False),
            keep_unused=True)
        self._fn = fn
        self._in_names = in_names
        self._out_names = out_names
        self._out_avals = out_avals
        self._zero_outs = zero_outs

    def prepare_args(self, in_maps):
        import jax
        NC = self.cfg["NC"]
        per_core = [[np.asarray(m[name]) for name in self._in_names]
                    for m in in_maps]
        concat_in = [np.concatenate([per_core[c][i] for c in range(NC)], axis=0)
                     for i in range(len(self._in_names))]
        concat_zeros = [np.zeros((NC * z.shape[0], *z.shape[1:]), z.dtype)
                        for z in self._zero_outs]
        return [jax.device_put(a) for a in concat_in + concat_zeros]

    def time_ns(self, in_maps, r_hi=6, reps=40):
        """Per-execution device time, measured by differencing wall times of
        this NEFF vs a variant whose body repeats the whole kernel r_hi times
        (fixed RPC + input-staging costs cancel in the difference)."""
        import time
        import jax

        if self._fn is None:
            self._build_fn()
        if not hasattr(self, "_fn_hi") or self._fn_hi is None:
            rh = _Runner.__new__(_Runner)
            rh.cfg = self.cfg
            rh.plan = self.plan
            rh.nc = build_program(self.plan, self.cfg, repeat=r_hi)
            rh._fn = None
            rh._build_fn()
            self._fn_hi = rh._fn
            self._rh = rh
            self._r_hi = r_hi
        a1 = self.prepare_args(in_maps)
        ah = self._rh.prepare_args(in_maps)
        jax.block_until_ready(self._fn(*a1))
        jax.block_until_ready(self._fn_hi(*ah))
        t1s, ths = [], []
        for _ in range(reps):
            t0 = time.perf_counter()
            jax.block_until_ready(self._fn(*a1))
            t1s.append(time.perf_counter() - t0)
            t0 = time.perf_counter()
            jax.block_until_ready(self._fn_hi(*ah))
            ths.append(time.perf_counter() - t0)
        t1s.sort()
        ths.sort()
        i = max(1, reps // 10)
        return (ths[i] - t1s[i]) / (self._r_hi - 1) * 1e9

    def run(self, in_maps):
        import jax
        if self._fn is None:
            self._build_fn()
        args = self.prepare_args(in_maps)
        outs = self._fn(*args)
        jax.block_until_ready(outs)
        NC = self.cfg["NC"]
        res = []
        for c in range(NC):
            m = {}
            for i, name in enumerate(self._out_names):
                m[name] = np.asarray(outs[i]).reshape(
                    NC, *self._out_avals[i].shape)[c]
            res.append(m)
        return res


def get_runner(inputs, cfg=None):
    cfg = dict(DEFAULT_CFG if cfg is None else cfg)
    plan = make_plan(np.asarray(inputs["edge_row"]).astype(np.int64),
                     np.asarray(inputs["edge_col"]).astype(np.int64), cfg)
    key = plan_key(plan, cfg)
    if key not in _CACHE:
        _CACHE[key] = _Runner(plan, cfg)
    return _CACHE[key], plan


def kernel(**inputs) -> np.ndarray:
    cfg = dict(DEFAULT_CFG)
    runner, plan = get_runner(inputs, cfg)
    in_maps = pack_inputs(inputs, runner.plan, cfg)
    res = runner.run(in_maps)
    NSH = runner.plan["d"]["NSH"]
    out = np.concatenate([res[k]["out"][:NSH] for k in range(cfg["NC"])],
                         axis=0)
    return out[:cfg["N"]].astype(np.float32)
